# revision 1
# baseline (speedup 1.0000x reference)
"""BiLSTM dual-pathway + CRF NLL kernel for 8 Trainium2 NeuronCores.

Sharding: data-parallel over batch (B=64 -> 8 per core). Each core runs the
full network on its batch shard entirely on-device and emits a partial sum of
(denom - num) over its 8 sequences; host sums and divides by 64.

On-chip layout is feature-major: features on SBUF partitions, (t*BL + b) on the
free axis. LSTM gate order is permuted to [i, f, o, g] so sigmoid applies to one
contiguous span. Matmul operands are fp16 (FWL weight loads), state/CRF fp32.
Backward-direction time reversal is done with negative-stride access patterns,
never materialized.
"""

import sys

sys.path.insert(0, "/opt/trn_rl_repo")

import numpy as np

import concourse.bass as bass
import concourse.mybir as mybir
from concourse import bacc
from concourse.bass import ds
from concourse.masks import make_identity
from concourse.tile import TileContext
from concourse.bass_utils import run_bass_kernel_spmd

F16 = mybir.dt.float16
F32 = mybir.dt.float32
AF = mybir.ActivationFunctionType

B, T, V, K = 64, 512, 40, 15
NC_N = 8
BL = B // NC_N          # 8 sequences per core
TB = T * BL             # 4096 free columns
UNROLL = 16

# directions: (name, Dk chunks of input, source kind, reverse, hs slot)
DIRS = [
    ("c0f", 1, "ce", False, "l0f"), ("c0b", 1, "ce", True, "l0b"),
    ("c1f", 4, "ch0", False, "c1f"), ("c1b", 4, "ch0", True, "c1b"),
    ("w0f", 6, "we", False, "l0f"), ("w0b", 6, "we", True, "l0b"),
    ("w1f", 4, "wh0", False, "w1f"), ("w1b", 4, "wh0", True, "w1b"),
]

_BUILD_CACHE = {}


def _seq_ap(tile, k, col_lo, n_steps, reverse):
    """AP over tile[:, k, :]: n_steps time-blocks of BL cols, fwd or reversed."""
    p_step = tile.ap[0][0]
    W = tile.ap[2][1]
    off = tile.offset + k * W + col_lo
    step = -BL if reverse else BL
    return bass.AP(tensor=tile.tensor, offset=off,
                   ap=[[p_step, 128], [step, n_steps], [1, BL]])


def _build_nc():
    if "nc" in _BUILD_CACHE:
        return _BUILD_CACHE["nc"]
    nc = bacc.Bacc(target_bir_lowering=False)

    # ---- external parameters -------------------------------------------------
    ceT_ext = nc.declare_dram_parameter("ceT", [128, 1, TB], F16, isOutput=False)
    weT_ext = nc.declare_dram_parameter("weT", [128, 6, TB], F16, isOutput=False)
    wih_ext, whh_ext = {}, {}
    for nm, dk, _, _, _ in DIRS:
        wih_ext[nm] = nc.declare_dram_parameter(f"wih_{nm}", [128, dk * 8 * 128], F16, isOutput=False)
        whh_ext[nm] = nc.declare_dram_parameter(f"whh_{nm}", [128, 2 * 8 * 128], F16, isOutput=False)
    biasall_ext = nc.declare_dram_parameter("biasall", [128, 8, 8], F32, isOutput=False)
    cls1_ext = nc.declare_dram_parameter("cls1", [128, 8 * 4 * 128], F16, isOutput=False)
    clsb1_ext = nc.declare_dram_parameter("clsb1", [128, 4], F32, isOutput=False)
    cls2_ext = nc.declare_dram_parameter("cls2", [128, 4 * 15], F16, isOutput=False)
    clsb2_ext = nc.declare_dram_parameter("clsb2", [15, 1], F32, isOutput=False)
    trans_ext = nc.declare_dram_parameter("trans", [15, 15], F32, isOutput=False)
    start_ext = nc.declare_dram_parameter("crfstart", [15, 1], F32, isOutput=False)
    end_ext = nc.declare_dram_parameter("crfend", [15, 1], F32, isOutput=False)
    tago_ext = nc.declare_dram_parameter("tagoneT", [15, TB], F16, isOutput=False)
    out_ext = nc.declare_dram_parameter("out", [1, 1], F32, isOutput=True)

    # ---- internal DRAM: pre-activation gate inputs per direction -------------
    xg_dram = {nm: nc.dram_tensor(f"xg_{nm}", [128, 8, TB], F16) for nm, _, _, _, _ in DIRS}

    with TileContext(nc) as tc:
        with (
            tc.tile_pool(name="consts", bufs=1) as consts,
            tc.tile_pool(name="seqs", bufs=1) as seqs,
            tc.tile_pool(name="wpool", bufs=1) as wpool,
            tc.tile_pool(name="work", bufs=2) as work,
            tc.tile_pool(name="stage", bufs=3) as stagep,
            tc.tile_pool(name="ps_big", bufs=2, space="PSUM") as ps_big,
            tc.tile_pool(name="ps_rec", bufs=2, space="PSUM") as ps_rec,
            tc.tile_pool(name="ps_small", bufs=3, space="PSUM") as ps_small,
        ):
            ident = consts.tile([128, 128], F32, tag="ident")
            make_identity(nc, ident)

            ceT = consts.tile([128, 1, TB], F16, tag="ceT")
            nc.sync.dma_start(out=ceT, in_=ceT_ext[:, :, :])
            cls1 = consts.tile([128, 8, 4, 128], F16, tag="cls1")
            nc.sync.dma_start(out=cls1, in_=cls1_ext.ap().rearrange("p (k m c) -> p k m c", k=8, m=4))
            clsb1 = consts.tile([128, 4], F32, tag="clsb1")
            nc.sync.dma_start(out=clsb1, in_=clsb1_ext[:, :])
            cls2 = consts.tile([128, 4, 15], F16, tag="cls2")
            nc.sync.dma_start(out=cls2, in_=cls2_ext.ap().rearrange("p (k j) -> p k j", k=4))
            clsb2 = consts.tile([15, 1], F32, tag="clsb2")
            nc.sync.dma_start(out=clsb2, in_=clsb2_ext[:, :])
            trans = consts.tile([15, 15], F32, tag="trans")
            nc.sync.dma_start(out=trans, in_=trans_ext[:, :])
            crfstart = consts.tile([15, 1], F32, tag="crfstart")
            nc.sync.dma_start(out=crfstart, in_=start_ext[:, :])
            crfend = consts.tile([15, 1], F32, tag="crfend")
            nc.sync.dma_start(out=crfend, in_=end_ext[:, :])
            tago = consts.tile([15, TB], F16, tag="tago")
            nc.sync.dma_start(out=tago, in_=tago_ext[:, :])
            biasall = consts.tile([128, 8, 8], F32, tag="biasall")
            nc.sync.dma_start(out=biasall, in_=biasall_ext[:, :, :])

            # sequence buffers: [128, 2, BL + T*BL] fp16; col (s+1)*BL holds h_s.
            # l0f/l0b slots are reused by the word pathway after the char L1
            # inputs have been consumed.
            hs_slot = {}
            for slot in ("l0f", "l0b", "c1f", "c1b", "w1f", "w1b"):
                hs_slot[slot] = seqs.tile([128, 2, BL + TB], F16, tag=f"hs_{slot}",
                                          name=f"hs_{slot}")
            hs = {nm: hs_slot[slot] for nm, _, _, _, slot in DIRS}

            def xg_rhs_ap(src_tile, dk, ns, reverse, width_steps, col_base):
                """rhs AP (128 x 64*BL) for xg matmul: source chunk dk, s-tile ns."""
                if not reverse:
                    return _seq_ap(src_tile, dk, col_base + ns * 64 * BL, 64, False)
                top = width_steps - 1 - ns * 64
                return _seq_ap(src_tile, dk, col_base + top * BL, 64, True)

            def xg_phase(di, nm, dk_n, src_kind, reverse):
                wih = wpool.tile([128, 6, 8, 128], F16, tag="wih")
                nc.sync.dma_start(
                    out=wih[:, :dk_n],
                    in_=wih_ext[nm].ap().rearrange("p (k m c) -> p k m c", k=dk_n, m=8),
                )
                for ns in range(8):
                    if src_kind == "we":
                        wxs = work.tile([128, 6, 64 * BL], F16, tag="wxs", bufs=1)
                        blk = (7 - ns) if reverse else ns
                        nc.sync.dma_start(out=wxs, in_=weT_ext[:, :, ds(blk * 64 * BL, 64 * BL)])
                    for m in range(8):
                        ps = ps_big.tile([128, 64, BL], F32, tag="xgps")
                        for dk in range(dk_n):
                            if src_kind == "ce":
                                rhs = xg_rhs_ap(ceT, dk, ns, reverse, T, 0)
                            elif src_kind == "we":
                                rhs = xg_rhs_ap(wxs, dk, 0, reverse, 64, 0)
                            else:
                                pre = "c0" if src_kind == "ch0" else "w0"
                                base = hs[pre + ("f" if dk < 2 else "b")]
                                krev = reverse if dk < 2 else (not reverse)
                                rhs = xg_rhs_ap(base, dk % 2, ns, krev, T, BL)
                            nc.tensor.matmul(ps, wih[:, dk, m], rhs,
                                             start=(dk == 0), stop=(dk == dk_n - 1))
                        st = stagep.tile([128, 64 * BL], F16, tag="xgstage")
                        nc.vector.tensor_scalar_add(st, ps, biasall[:, di, m : m + 1])
                        nc.sync.dma_start(out=xg_dram[nm][:, m, ds(ns * 64 * BL, 64 * BL)], in_=st)

            def lstm_rec(nm):
                hst = hs[nm]
                whh = work.tile([128, 2, 8, 128], F16, tag="whh")
                nc.sync.dma_start(
                    out=whh, in_=whh_ext[nm].ap().rearrange("p (k m c) -> p k m c", k=2, m=8)
                )
                cst = work.tile([128, 2, BL], F32, tag="cstate")
                nc.vector.memset(cst, 0.0)
                nc.vector.memset(hst[:, :, 0:BL], 0.0)
                with tc.For_i(0, T, UNROLL) as tv:
                    xgs = stagep.tile([128, 8, UNROLL * BL], F16, tag="xgs")
                    nc.sync.dma_start(out=xgs, in_=xg_dram[nm][:, :, ds(tv * BL, UNROLL * BL)])
                    for j in range(UNROLL):
                        ps = ps_rec.tile([128, 8, BL], F32, tag="recps")
                        for m in range(8):
                            for k in range(2):
                                nc.tensor.matmul(
                                    ps[:, m], whh[:, k, m],
                                    hst[:, k, ds(tv * BL + j * BL, BL)],
                                    start=(k == 0), stop=(k == 1),
                                )
                        g = stagep.tile([128, 8, BL], F32, tag="g")
                        nc.vector.tensor_add(g, ps, xgs[:, :, j * BL : (j + 1) * BL])
                        sig = stagep.tile([128, 6, BL], F32, tag="sig")
                        nc.scalar.activation(sig, g[:, 0:6], AF.Sigmoid)
                        tgg = stagep.tile([128, 2, BL], F32, tag="tgg")
                        nc.scalar.activation(tgg, g[:, 6:8], AF.Tanh)
                        tmp = stagep.tile([128, 2, BL], F32, tag="tmpig")
                        nc.vector.tensor_mul(tmp, sig[:, 0:2], tgg)
                        nc.vector.tensor_mul(cst, cst, sig[:, 2:4])
                        nc.vector.tensor_add(cst, cst, tmp)
                        tch = stagep.tile([128, 2, BL], F32, tag="tch")
                        nc.scalar.activation(tch, cst, AF.Tanh)
                        nc.vector.tensor_mul(
                            hst[:, :, ds(tv * BL + j * BL + BL, BL)], sig[:, 4:6], tch
                        )

            for di, (nm, dk_n, src, rev, _) in enumerate(DIRS):
                xg_phase(di, nm, dk_n, src, rev)
                lstm_rec(nm)

            # ---- classifier + logits --------------------------------------
            logits = seqs.tile([15, TB], F32, tag="logits")

            def comb_rhs(kk, ns):
                names = ["c1f", "c1b", "w1f", "w1b"]
                base = hs[names[kk // 2]]
                rev = (kk // 2) % 2 == 1
                return xg_rhs_ap(base, kk % 2, ns, rev, T, BL)

            for ns in range(8):
                hmt = []
                for m in range(4):
                    ps = ps_big.tile([128, 64, BL], F32, tag="xgps")
                    for kk in range(8):
                        nc.tensor.matmul(ps, cls1[:, kk, m], comb_rhs(kk, ns),
                                         start=(kk == 0), stop=(kk == 7))
                    hm = stagep.tile([128, 64 * BL], F16, tag="hm", bufs=4, name=f"hm{m}")
                    nc.scalar.activation(hm, ps, AF.Relu, bias=clsb1[:, m : m + 1])
                    hmt.append(hm)
                ps2 = ps_small.tile([15, 64 * BL], F32, tag="small")
                for m in range(4):
                    nc.tensor.matmul(ps2, cls2[:, m], hmt[m], start=(m == 0), stop=(m == 3))
                nc.vector.tensor_scalar_add(logits[:, ds(ns * 64 * BL, 64 * BL)], ps2, clsb2)

            # fold CRF start/end into first/last emission columns
            nc.vector.tensor_scalar_add(logits[:, 0:BL], logits[:, 0:BL], crfstart)
            nc.vector.tensor_scalar_add(logits[:, TB - BL : TB], logits[:, TB - BL : TB], crfend)

            # ---- CRF numerator --------------------------------------------
            # emission part: sum(logits * onehot); transition part via
            # V = trans^T @ onehot, shifted dot with onehot.
            racc = work.tile([15, 16], F32, tag="racc")
            nc.vector.memset(racc, 0.0)
            trans16 = consts.tile([15, 15], F16, tag="trans16")
            nc.vector.tensor_copy(trans16, trans)
            for ns in range(8):
                psv = ps_small.tile([15, 64 * BL], F32, tag="small")
                nc.tensor.matmul(psv, trans16, tago[:, ds(ns * 64 * BL, 64 * BL)], start=True, stop=True)
                w = 64 * BL if ns < 7 else 64 * BL - BL
                pr = stagep.tile([15, 64 * BL], F32, tag="prodns")
                nc.vector.tensor_mul(pr[:, :w], psv[:, :w], tago[:, ds(ns * 64 * BL + BL, w)])
                nc.vector.tensor_reduce(racc[:, ns : ns + 1], pr[:, :w],
                                        axis=mybir.AxisListType.X, op=mybir.AluOpType.add)
                pr2 = stagep.tile([15, 64 * BL], F32, tag="prodns")
                nc.vector.tensor_mul(pr2, logits[:, ds(ns * 64 * BL, 64 * BL)],
                                     tago[:, ds(ns * 64 * BL, 64 * BL)])
                nc.vector.tensor_reduce(racc[:, 8 + ns : 9 + ns], pr2,
                                        axis=mybir.AxisListType.X, op=mybir.AluOpType.add)
            nv = stagep.tile([15, 1], F32, tag="nv")
            nc.vector.tensor_reduce(nv, racc, axis=mybir.AxisListType.X, op=mybir.AluOpType.add)
            ones15 = consts.tile([15, 1], F32, tag="ones15")
            nc.vector.memset(ones15, 1.0)
            psn = ps_small.tile([1, 1], F32, tag="small")
            nc.tensor.matmul(psn, ones15, nv, start=True, stop=True)
            num11 = work.tile([1, 1], F32, tag="num11")
            nc.vector.tensor_copy(num11, psn)

            # ---- CRF forward scan (layout: [tags(15) x batch(8)]) ----------
            mexp = consts.tile([15, 15], F32, tag="mexp")
            nc.scalar.activation(mexp, trans, AF.Exp)
            ones115 = consts.tile([1, 15], F32, tag="ones115")
            nc.vector.memset(ones115, 1.0)
            alpha = seqs.tile([15, BL], F32, tag="alpha")
            nc.vector.tensor_copy(alpha, logits[:, 0:BL])
            off_r = seqs.tile([1, BL], F32, tag="off_r")
            nc.vector.memset(off_r, 0.0)

            def crf_step(e_ap):
                p8 = stagep.tile([15, BL], F32, tag="crfp")
                nc.scalar.activation(p8, alpha, AF.Exp)
                z = ps_small.tile([15, BL], F32, tag="small")
                nc.tensor.matmul(z, mexp, p8, start=True, stop=True)
                lnz = stagep.tile([15, BL], F32, tag="crflnz")
                nc.scalar.activation(lnz, z, AF.Ln)
                nc.vector.tensor_add(alpha, lnz, e_ap)

            def crf_renorm():
                pt = ps_small.tile([BL, 15], F32, tag="small")
                nc.tensor.transpose(pt, alpha, ident[:15, :15])
                a8 = stagep.tile([BL, 15], F32, tag="crfa8")
                nc.vector.tensor_copy(a8, pt)
                negm = stagep.tile([BL, 1], F32, tag="crfnegm")
                nc.vector.tensor_reduce(negm, a8, axis=mybir.AxisListType.X,
                                        op=mybir.AluOpType.max, negate=True)
                ptm = ps_small.tile([1, BL], F32, tag="small")
                nc.tensor.transpose(ptm, negm, ident[:BL, :BL])
                nr = stagep.tile([1, BL], F32, tag="crfnr")
                nc.vector.tensor_copy(nr, ptm)
                bps = ps_small.tile([15, BL], F32, tag="small")
                nc.tensor.matmul(bps, ones115, nr, start=True, stop=True)
                nc.vector.tensor_add(alpha, alpha, bps)
                nc.vector.tensor_sub(off_r, off_r, nr)

            with tc.For_i(0, 496, UNROLL) as tv:
                for j in range(UNROLL):
                    crf_step(logits[:, ds(tv * BL + (j + 1) * BL, BL)])
                crf_renorm()
            for t in range(497, T):
                crf_step(logits[:, t * BL : (t + 1) * BL])

            # ---- denominator + output -------------------------------------
            ptf = ps_small.tile([BL, 15], F32, tag="small")
            nc.tensor.transpose(ptf, alpha, ident[:15, :15])
            af8 = stagep.tile([BL, 15], F32, tag="af8")
            nc.vector.tensor_copy(af8, ptf)
            negm2 = stagep.tile([BL, 1], F32, tag="negm2")
            nc.vector.tensor_reduce(negm2, af8, axis=mybir.AxisListType.X,
                                    op=mybir.AluOpType.max, negate=True)
            e8 = stagep.tile([BL, 15], F32, tag="e8")
            s8 = stagep.tile([BL, 1], F32, tag="s8")
            nc.scalar.activation(e8, af8, AF.Exp, bias=negm2, accum_out=s8)
            l8 = stagep.tile([BL, 1], F32, tag="l8")
            nc.scalar.activation(l8, s8, AF.Ln)
            den8 = stagep.tile([BL, 1], F32, tag="den8")
            nc.vector.tensor_sub(den8, l8, negm2)
            pso = ps_small.tile([BL, 1], F32, tag="small")
            nc.tensor.transpose(pso, off_r, ident[:1, :1])
            o8 = stagep.tile([BL, 1], F32, tag="o8")
            nc.vector.tensor_copy(o8, pso)
            nc.vector.tensor_add(den8, den8, o8)
            ones8 = consts.tile([BL, 1], F32, tag="ones8")
            nc.vector.memset(ones8, 1.0)
            psd = ps_small.tile([1, 1], F32, tag="small")
            nc.tensor.matmul(psd, ones8, den8, start=True, stop=True)
            den11 = work.tile([1, 1], F32, tag="den11")
            nc.vector.tensor_copy(den11, psd)
            res = work.tile([1, 1], F32, tag="res")
            nc.vector.tensor_sub(res, den11, num11)
            nc.sync.dma_start(out=out_ext[:, :], in_=res)

    nc.finalize()
    _BUILD_CACHE["nc"] = nc
    return nc


# ---- host-side input prep ---------------------------------------------------

_GPERM = np.concatenate([np.arange(0, 512), np.arange(768, 1024), np.arange(512, 768)])


def _wih_prep(W, dk_n):
    # lhsT tiles: [p, dk, m, c] = W[gperm[m*128+c], dk*128+p]
    Wp = W[_GPERM]
    return np.ascontiguousarray(
        Wp.reshape(8, 128, dk_n, 128).transpose(3, 2, 0, 1).reshape(128, dk_n * 8 * 128)
    ).astype(np.float16)


def _common_inputs(inputs):
    out = {}
    bias_cols = []
    for key in ("c0", "c1", "w0", "w1"):
        Wih = np.asarray(inputs[f"{key}_Wih"], np.float32)
        Whh = np.asarray(inputs[f"{key}_Whh"], np.float32)
        bih = np.asarray(inputs[f"{key}_bih"], np.float32)
        bhh = np.asarray(inputs[f"{key}_bhh"], np.float32)
        dk_n = Wih.shape[2] // 128
        for r, sfx in ((0, "f"), (1, "b")):
            out[f"wih_{key}{sfx}"] = _wih_prep(Wih[r], dk_n)
            out[f"whh_{key}{sfx}"] = _wih_prep(Whh[r], 2)
            bb = (bih[r] + bhh[r])[_GPERM]
            bias_cols.append(bb.reshape(8, 128).T)  # (128, 8)
    # DIRS order is c0f,c0b,c1f,c1b,w0f,w0b,w1f,w1b == bias_cols order
    out["biasall"] = np.ascontiguousarray(np.stack(bias_cols, axis=1)).astype(np.float32)
    w1 = np.asarray(inputs["cls_w1"], np.float32)  # (512, 1024)
    out["cls1"] = np.ascontiguousarray(
        w1.reshape(4, 128, 8, 128).transpose(3, 2, 0, 1).reshape(128, 8 * 4 * 128)
    ).astype(np.float16)
    out["clsb1"] = np.ascontiguousarray(
        np.asarray(inputs["cls_b1"], np.float32).reshape(4, 128).T
    ).astype(np.float32)
    w2 = np.asarray(inputs["cls_w2"], np.float32)  # (15, 512)
    out["cls2"] = np.ascontiguousarray(
        w2.reshape(15, 4, 128).transpose(2, 1, 0).reshape(128, 4 * 15)
    ).astype(np.float16)
    out["clsb2"] = np.asarray(inputs["cls_b2"], np.float32).reshape(15, 1).copy()
    out["trans"] = np.asarray(inputs["crf_trans"], np.float32).copy()
    out["crfstart"] = np.asarray(inputs["crf_start"], np.float32).reshape(15, 1).copy()
    out["crfend"] = np.asarray(inputs["crf_end"], np.float32).reshape(15, 1).copy()
    return out


def kernel(**inputs):
    nc = _build_nc()
    common = _common_inputs(inputs)
    char_ids = np.asarray(inputs["char_ids"])
    tags = np.asarray(inputs["tags"])
    wemb = np.asarray(inputs["word_embeddings"], np.float32)
    emb = np.asarray(inputs["char_emb_table"], np.float32)

    in_maps = []
    for c in range(NC_N):
        lo, hi = c * BL, (c + 1) * BL
        m = dict(common)
        ce = emb[char_ids[lo:hi]]  # (BL, T, 128)
        m["ceT"] = np.ascontiguousarray(
            ce.transpose(2, 1, 0).reshape(128, 1, TB)
        ).astype(np.float16)
        m["weT"] = np.ascontiguousarray(
            wemb[lo:hi].reshape(BL, T, 6, 128).transpose(3, 2, 1, 0).reshape(128, 6, TB)
        ).astype(np.float16)
        oh = (np.arange(K)[:, None, None] == tags[lo:hi][None]).astype(np.float32)
        m["tagoneT"] = np.ascontiguousarray(oh.transpose(0, 2, 1).reshape(K, TB)).astype(np.float16)
        in_maps.append(m)

    res = run_bass_kernel_spmd(nc, in_maps, core_ids=list(range(NC_N)))
    total = sum(float(res.results[c]["out"][0, 0]) for c in range(NC_N))
    return np.float32(total / B)



# revision 2
# speedup vs baseline: 1.2937x; 1.2937x over previous
"""BiLSTM dual-pathway + CRF NLL kernel for 8 Trainium2 NeuronCores (V2).

Sharding: data-parallel over batch (B=64 -> 8 per core). Each core runs the
full network on its batch shard and emits a partial sum of (denom - num) over
its 8 sequences; host sums and divides by 64.

V3 structure: the 8 LSTM directions run as 2 waves of 4 concurrent
recurrences (wave1: c0f,c0b,w0f,w0b; wave2: c1f,c1b,w1f,w1b), interleaved
step-by-step inside one hardware time loop so the PE array stays warm and the
vector/scalar gate chain hides under the next directions' matmuls. The CRF
forward scan runs in linear space (p' = E_t o (M^T p), M = exp(trans)/16)
with renormalization every 64 steps, which is 2 small ops per step.

On-chip layout is feature-major: features on SBUF partitions, (t*BL + b) on
the free axis. LSTM gate order is permuted to [i, f, o, g]. Matmul operands
are fp16; state/CRF fp32. Backward-direction time reversal uses
negative-stride access patterns.
"""

import sys

sys.path.insert(0, "/opt/trn_rl_repo")

import numpy as np

import concourse.bass as bass
import concourse.mybir as mybir
from concourse import bacc
from concourse.bass import ds
from concourse.masks import make_identity
from concourse.tile import TileContext
from concourse.bass_utils import run_bass_kernel_spmd

F16 = mybir.dt.float16
F32 = mybir.dt.float32
AF = mybir.ActivationFunctionType
AX = mybir.AxisListType
OP = mybir.AluOpType

B, T, V, K = 64, 512, 40, 15
NC_N = 8
BL = B // NC_N          # 8 sequences per core
TB = T * BL             # 4096 free columns
UNROLL = 16
LOG16 = float(np.log(16.0))

# waves of 4 directions: (name, Dk chunks, source kind, reverse)
WAVE1 = [("c0f", 1, "ce", False), ("c0b", 1, "ce", True),
         ("w0f", 6, "we", False), ("w0b", 6, "we", True)]
WAVE2 = [("c1f", 4, "c0", False), ("c1b", 4, "c0", True),
         ("w1f", 4, "w0", False), ("w1b", 4, "w0", True)]
DIR_ORDER = [nm for nm, _, _, _ in WAVE1 + WAVE2]

_BUILD_CACHE = {}


def _seq_ap(tile, k, col_lo, n_steps, reverse):
    """AP over per-dir tile[:, k, :]: n_steps blocks of BL cols, fwd or rev."""
    p_step = tile.ap[0][0]
    Wd = tile.ap[2][1]          # cols per k row
    off = tile.offset + k * Wd + col_lo
    step = -BL if reverse else BL
    return bass.AP(tensor=tile.tensor, offset=off,
                   ap=[[p_step, 128], [step, n_steps], [1, BL]])


def _ps_chain_ap(ps, d):
    """[128, p(2), t(4), b(8)] view of a group psum tile for direction d."""
    p_step = ps.ap[0][0]
    return bass.AP(tensor=ps.tensor, offset=ps.offset + d * 8,
                   ap=[[p_step, 128], [512, 2], [16, 4], [1, 8]])


def _xg_chain_ap(xgt, j):
    """[128, p(2), t(4), b(8)] view of xg stage tile [128, 8, U*BL] at step j."""
    p_step = xgt.ap[0][0]
    mstride = xgt.ap[1][0]      # U*BL
    return bass.AP(tensor=xgt.tensor, offset=xgt.offset + j * BL,
                   ap=[[p_step, 128], [mstride, 2], [2 * mstride, 4], [1, 8]])


def _build_nc():
    if "nc" in _BUILD_CACHE:
        return _BUILD_CACHE["nc"]
    nc = bacc.Bacc(target_bir_lowering=False)

    # ---- external parameters -------------------------------------------------
    ceT_ext = nc.declare_dram_parameter("ceT", [128, 1, TB], F16, isOutput=False)
    weT_ext = nc.declare_dram_parameter("weT", [128, 6, TB], F16, isOutput=False)
    wih_ext, whh_ext = {}, {}
    for nm, dk, _, _ in WAVE1 + WAVE2:
        wih_ext[nm] = nc.declare_dram_parameter(f"wih_{nm}", [128, dk * 8 * 128], F16, isOutput=False)
        whh_ext[nm] = nc.declare_dram_parameter(f"whh_{nm}", [128, 2 * 8 * 128], F16, isOutput=False)
    biasall_ext = nc.declare_dram_parameter("biasall", [128, 8, 8], F32, isOutput=False)
    cls1_ext = nc.declare_dram_parameter("cls1", [128, 8 * 4 * 128], F16, isOutput=False)
    clsb1_ext = nc.declare_dram_parameter("clsb1", [128, 4], F32, isOutput=False)
    cls2_ext = nc.declare_dram_parameter("cls2", [128, 4 * 15], F16, isOutput=False)
    clsb2_ext = nc.declare_dram_parameter("clsb2", [15, 1], F32, isOutput=False)
    trans_ext = nc.declare_dram_parameter("trans", [15, 15], F32, isOutput=False)
    start_ext = nc.declare_dram_parameter("crfstart", [15, 1], F32, isOutput=False)
    end_ext = nc.declare_dram_parameter("crfend", [15, 1], F32, isOutput=False)
    tago_ext = nc.declare_dram_parameter("tagoneT", [15, TB], F16, isOutput=False)
    out_ext = nc.declare_dram_parameter("out", [1, 1], F32, isOutput=True)

    # internal DRAM: pre-activation gate inputs, one slot per wave position
    xg_dram = [nc.dram_tensor(f"xg_{i}", [128, 8, TB], F16) for i in range(4)]

    with TileContext(nc) as tc:
        with (
            tc.tile_pool(name="consts", bufs=1) as consts,
            tc.tile_pool(name="seqs", bufs=1) as seqs,
            tc.tile_pool(name="wpool", bufs=1) as wpool,
            tc.tile_pool(name="work", bufs=2) as work,
            tc.tile_pool(name="stage", bufs=3) as stagep,
            tc.tile_pool(name="ps_big", bufs=2, space="PSUM") as ps_big,
            tc.tile_pool(name="ps_rec", bufs=2, space="PSUM") as ps_rec,
            tc.tile_pool(name="ps_small", bufs=2, space="PSUM") as ps_small,
        ):
            ident = consts.tile([128, 128], F32, tag="ident")
            make_identity(nc, ident)

            ceT = consts.tile([128, 1, TB], F16, tag="ceT")
            nc.sync.dma_start(out=ceT, in_=ceT_ext[:, :, :])
            cls1 = consts.tile([128, 8, 4, 128], F16, tag="cls1")
            nc.sync.dma_start(out=cls1, in_=cls1_ext.ap().rearrange("p (k m c) -> p k m c", k=8, m=4))
            clsb1 = consts.tile([128, 4], F32, tag="clsb1")
            nc.sync.dma_start(out=clsb1, in_=clsb1_ext[:, :])
            cls2 = consts.tile([128, 4, 15], F16, tag="cls2")
            nc.sync.dma_start(out=cls2, in_=cls2_ext.ap().rearrange("p (k j) -> p k j", k=4))
            clsb2 = consts.tile([15, 1], F32, tag="clsb2")
            nc.sync.dma_start(out=clsb2, in_=clsb2_ext[:, :])
            trans = consts.tile([15, 15], F32, tag="trans")
            nc.sync.dma_start(out=trans, in_=trans_ext[:, :])
            crfstart = consts.tile([15, 1], F32, tag="crfstart")
            nc.sync.dma_start(out=crfstart, in_=start_ext[:, :])
            crfend = consts.tile([15, 1], F32, tag="crfend")
            nc.sync.dma_start(out=crfend, in_=end_ext[:, :])
            tago = consts.tile([15, TB], F16, tag="tago")
            nc.sync.dma_start(out=tago, in_=tago_ext[:, :])
            biasall = consts.tile([128, 8, 8], F32, tag="biasall")
            nc.sync.dma_start(out=biasall, in_=biasall_ext[:, :, :])

            # per-direction h-sequence buffers: [128, k(2), BL + T*BL] fp16;
            # col (s+1)*BL holds h_s in scan order (b-dirs stored reversed).
            # wave-1 buffers are reused by the matching wave-2 direction.
            hs = {}
            for slot in ("cf", "cb", "wf", "wb"):
                hs[slot] = seqs.tile([128, 2, BL + TB], F16, tag=f"hs_{slot}",
                                     name=f"hs_{slot}")
            slot_of = {nm: nm[0] + nm[2] for nm in DIR_ORDER}

            def xg_rhs_ap(src_tile, k, ns, reverse, width_steps, col_base):
                if not reverse:
                    return _seq_ap(src_tile, k, col_base + ns * 64 * BL, 64, False)
                top = width_steps - 1 - ns * 64
                return _seq_ap(src_tile, k, col_base + top * BL, 64, True)

            def xg_phase(slot, di, nm, dk_n, src_kind, reverse):
                """Gate-input matmuls for one direction -> xg_dram[slot]."""
                wih = wpool.tile([128, 6, 8, 128], F16, tag="wih", bufs=2)
                nc.sync.dma_start(
                    out=wih[:, :dk_n],
                    in_=wih_ext[nm].ap().rearrange("p (k m c) -> p k m c", k=dk_n, m=8),
                )
                for ns in range(8):
                    if src_kind == "we":
                        wxs = work.tile([128, 6, 64 * BL], F16, tag="wxs", bufs=2)
                        blk = (7 - ns) if reverse else ns
                        nc.sync.dma_start(out=wxs, in_=weT_ext[:, :, ds(blk * 64 * BL, 64 * BL)])
                    for m in range(8):
                        ps = ps_big.tile([128, 64, BL], F32, tag="xgps")
                        for dk in range(dk_n):
                            if src_kind == "ce":
                                # ceT is [128, 1, TB]; treat as k=dk? only dk=0
                                p_step = ceT.ap[0][0]
                                off = ceT.offset + (0 if not reverse else 0)
                                if not reverse:
                                    rhs = bass.AP(tensor=ceT.tensor,
                                                  offset=ceT.offset + ns * 64 * BL,
                                                  ap=[[p_step, 128], [BL, 64], [1, BL]])
                                else:
                                    top = T - 1 - ns * 64
                                    rhs = bass.AP(tensor=ceT.tensor,
                                                  offset=ceT.offset + top * BL,
                                                  ap=[[p_step, 128], [-BL, 64], [1, BL]])
                            elif src_kind == "we":
                                p_step = wxs.ap[0][0]
                                W = wxs.ap[2][1]
                                if not reverse:
                                    rhs = bass.AP(tensor=wxs.tensor,
                                                  offset=wxs.offset + dk * W,
                                                  ap=[[p_step, 128], [BL, 64], [1, BL]])
                                else:
                                    rhs = bass.AP(tensor=wxs.tensor,
                                                  offset=wxs.offset + dk * W + 63 * BL,
                                                  ap=[[p_step, 128], [-BL, 64], [1, BL]])
                            else:
                                # layer-1 input: concat(fwd h, bwd h) of layer 0
                                pre = "c" if src_kind == "c0" else "w"
                                d_src = dk // 2          # 0 = fwd dir, 1 = bwd dir
                                base = hs[pre + ("f" if d_src == 0 else "b")]
                                k_src = dk % 2
                                krev = reverse if d_src == 0 else (not reverse)
                                rhs = xg_rhs_ap(base, k_src, ns, krev, T, BL)
                            nc.tensor.matmul(ps, wih[:, dk, m], rhs,
                                             start=(dk == 0), stop=(dk == dk_n - 1))
                        st = stagep.tile([128, 64 * BL], F16, tag="xgstage")
                        nc.vector.tensor_scalar_add(st, ps, biasall[:, di, m : m + 1])
                        nc.sync.dma_start(out=xg_dram[slot][:, m, ds(ns * 64 * BL, 64 * BL)], in_=st)

            def rec_wave(wave, whh_tiles):
                """One wave: 4 directions (2 groups of f/b) interleaved in time.

                Per group one psum tile [128, 2, 512] spanning 2 banks; gate
                tile m goes to bank m%2 at col (m//2)*16 + d*8, so consecutive
                matmul accumulation groups alternate banks (overlapping one
                matmul's drain with the next one's fill).
                """
                hts = [[hs[slot_of[wave[2 * gi + d][0]]] for d in range(2)]
                       for gi in range(2)]
                csts = []
                for gi in range(2):
                    dirc = []
                    for d in range(2):
                        cst = work.tile([128, 2, BL], F32, tag=f"cst{gi}{d}", bufs=1,
                                        name=f"cst{gi}{d}")
                        nc.vector.memset(cst, 0.0)
                        nc.vector.memset(hts[gi][d][:, :, 0:BL], 0.0)
                        dirc.append(cst)
                    csts.append(dirc)
                with tc.For_i(0, T, UNROLL) as tv:
                    xgs = []
                    for di4 in range(4):
                        xgt = stagep.tile([128, 8, UNROLL * BL], F16, tag=f"xgs{di4}",
                                          name=f"xgs{di4}", bufs=2)
                        nc.sync.dma_start(out=xgt, in_=xg_dram[di4][:, :, ds(tv * BL, UNROLL * BL)])
                        xgs.append(xgt)
                    for j in range(UNROLL):
                        for gi in range(2):
                            whh = whh_tiles[gi]
                            ps = ps_rec.tile([128, 2, 512], F32, tag=f"recps{gi}",
                                             name=f"recps{gi}", bufs=1)
                            for d in range(2):
                                hst = hts[gi][d]
                                for m in range(8):
                                    out = ps[:, m % 2, ds((m // 2) * 16 + d * 8, BL)]
                                    for k in range(2):
                                        nc.tensor.matmul(
                                            out, whh[:, d, k, m],
                                            hst[:, k, ds(tv * BL + j * BL, BL)],
                                            start=(k == 0), stop=(k == 1),
                                        )
                            for d in range(2):
                                hst, cst = hts[gi][d], csts[gi][d]
                                g = stagep.tile([128, 2, 4, BL], F32, tag=f"g{gi}{d}",
                                                name=f"g{gi}{d}")
                                nc.vector.tensor_add(g, _ps_chain_ap(ps, d),
                                                     _xg_chain_ap(xgs[2 * gi + d], j))
                                sig = stagep.tile([128, 2, 3, BL], F32, tag=f"sig{gi}{d}",
                                                  name=f"sig{gi}{d}")
                                nc.scalar.activation(sig, g[:, :, 0:3], AF.Sigmoid)
                                tgg = stagep.tile([128, 2, BL], F32, tag=f"tgg{gi}{d}",
                                                  name=f"tgg{gi}{d}")
                                nc.scalar.activation(tgg, g[:, :, 3:4].squeeze(), AF.Tanh)
                                tmp = stagep.tile([128, 2, BL], F32, tag=f"tmp{gi}{d}",
                                                  name=f"tmp{gi}{d}")
                                nc.vector.tensor_mul(tmp, sig[:, :, 0:1].squeeze(), tgg)
                                nc.vector.tensor_mul(cst, cst, sig[:, :, 1:2].squeeze())
                                nc.vector.tensor_add(cst, cst, tmp)
                                tch = stagep.tile([128, 2, BL], F32, tag=f"tch{gi}{d}",
                                                  name=f"tch{gi}{d}")
                                nc.scalar.activation(tch, cst, AF.Tanh)
                                nc.vector.tensor_mul(
                                    hst[:, :, ds(tv * BL + j * BL + BL, BL)],
                                    sig[:, :, 2:3].squeeze(), tch)

            def load_whh(wave):
                tiles = []
                for gi in range(2):
                    whh = wpool.tile([128, 2, 2, 8, 128], F16, tag=f"whh{gi}",
                                     name=f"whh{gi}")
                    for d in range(2):
                        nm = wave[2 * gi + d][0]
                        nc.sync.dma_start(
                            out=whh[:, d],
                            in_=whh_ext[nm].ap().rearrange("p (k m c) -> p k m c", k=2, m=8))
                    tiles.append(whh)
                return tiles

            # ---- wave 1 ----------------------------------------------------
            for slot, (nm, dk_n, src, rev) in enumerate(WAVE1):
                xg_phase(slot, DIR_ORDER.index(nm), nm, dk_n, src, rev)
            rec_wave(WAVE1, load_whh(WAVE1))

            # ---- wave 2 ----------------------------------------------------
            for slot, (nm, dk_n, src, rev) in enumerate(WAVE2):
                xg_phase(slot, DIR_ORDER.index(nm), nm, dk_n, src, rev)
            rec_wave(WAVE2, load_whh(WAVE2))

            # ---- classifier + CRF inputs ----------------------------------
            # comb chunk kk -> (group, d, k): [c1f k0,k1, c1b k0,k1, w1f.., w1b..]
            Et = seqs.tile([15, TB], F16, tag="Et")            # exp(logits)

            racc = work.tile([15, 16], F32, tag="racc", bufs=1)
            nc.vector.memset(racc, 0.0)
            trans16 = consts.tile([15, 15], F16, tag="trans16")
            nc.vector.tensor_copy(trans16, trans)

            def comb_rhs(kk, ns):
                names = ["cf", "cb", "wf", "wb"]
                base = hs[names[kk // 2]]
                rev = (kk // 2) % 2 == 1
                return xg_rhs_ap(base, kk % 2, ns, rev, T, BL)

            for ns in range(8):
                hmt = []
                for m in range(4):
                    ps = ps_big.tile([128, 64, BL], F32, tag="xgps")
                    for kk in range(8):
                        nc.tensor.matmul(ps, cls1[:, kk, m], comb_rhs(kk, ns),
                                         start=(kk == 0), stop=(kk == 7))
                    hm = stagep.tile([128, 64 * BL], F16, tag="hm", bufs=4, name=f"hm{m}")
                    nc.scalar.activation(hm, ps, AF.Relu, bias=clsb1[:, m : m + 1])
                    hmt.append(hm)
                ps2 = ps_small.tile([15, 64 * BL], F32, tag="small")
                for m in range(4):
                    nc.tensor.matmul(ps2, cls2[:, m], hmt[m], start=(m == 0), stop=(m == 3))
                lg = stagep.tile([15, 64 * BL], F32, tag="lgits")
                nc.vector.tensor_scalar_add(lg, ps2, clsb2)
                if ns == 0:
                    nc.vector.tensor_scalar_add(lg[:, 0:BL], lg[:, 0:BL], crfstart)
                if ns == 7:
                    nc.vector.tensor_scalar_add(lg[:, 64 * BL - BL :], lg[:, 64 * BL - BL :], crfend)
                nc.scalar.activation(Et[:, ds(ns * 64 * BL, 64 * BL)], lg, AF.Exp)

                # CRF numerator pieces on this tile
                psv = ps_small.tile([15, 64 * BL], F32, tag="small")
                nc.tensor.matmul(psv, trans16, tago[:, ds(ns * 64 * BL, 64 * BL)], start=True, stop=True)
                w = 64 * BL if ns < 7 else 64 * BL - BL
                pr = stagep.tile([15, 64 * BL], F32, tag="prodns")
                nc.vector.tensor_mul(pr[:, :w], psv[:, :w], tago[:, ds(ns * 64 * BL + BL, w)])
                nc.vector.tensor_reduce(racc[:, ns : ns + 1], pr[:, :w],
                                        axis=AX.X, op=OP.add)
                pr2 = stagep.tile([15, 64 * BL], F32, tag="prodns")
                nc.vector.tensor_mul(pr2, lg, tago[:, ds(ns * 64 * BL, 64 * BL)])
                nc.vector.tensor_reduce(racc[:, 8 + ns : 9 + ns], pr2,
                                        axis=AX.X, op=OP.add)

            nv = stagep.tile([15, 1], F32, tag="nv")
            nc.vector.tensor_reduce(nv, racc, axis=AX.X, op=OP.add)
            ones15 = consts.tile([15, 1], F32, tag="ones15")
            nc.vector.memset(ones15, 1.0)
            psn = ps_small.tile([15, 64 * BL], F32, tag="small")
            nc.tensor.matmul(psn[:1, :1], ones15, nv, start=True, stop=True)
            num11 = work.tile([1, 1], F32, tag="num11", bufs=1)
            nc.vector.tensor_copy(num11, psn[:1, :1])

            # ---- CRF forward scan, linear space ---------------------------
            # p_t = E_t o (Mexp^T p_{t-1}), Mexp = exp(trans)/16 (fp32 matmul).
            # logZ = ln(sum_j p_T) + 511*ln16 + renorm offsets.
            mexp = consts.tile([15, 15], F32, tag="mexp")
            nlog16 = consts.tile([15, 1], F32, tag="nlog16")
            nc.vector.memset(nlog16, -LOG16)
            nc.scalar.activation(mexp, trans, AF.Exp, bias=nlog16)
            ones115 = consts.tile([1, 15], F32, tag="ones115")
            nc.vector.memset(ones115, 1.0)

            # start/end potentials are already folded into logits (hence Et)
            p0 = seqs.tile([15, BL], F32, tag="p0")
            nc.vector.tensor_copy(p0, Et[:, 0:BL])
            off_r = seqs.tile([1, BL], F32, tag="off_r")
            nc.vector.memset(off_r, 0.0)

            pcur = [p0]

            def crf_step(s):
                z = ps_small.tile([15, 64 * BL], F32, tag="small")
                nc.tensor.matmul(z[:, 0:BL], mexp, pcur[0], start=True, stop=True)
                pn = stagep.tile([15, BL], F32, tag="pnew", bufs=4)
                nc.vector.tensor_mul(pn, z[:, 0:BL], Et[:, ds(s * BL, BL)])
                pcur[0] = pn

            def crf_renorm():
                # per-batch max over states -> scale p to ~1, accumulate log
                pt = ps_small.tile([15, 64 * BL], F32, tag="small")
                nc.tensor.transpose(pt[:BL, :15], pcur[0], ident[:15, :15])
                a8 = stagep.tile([BL, 15], F32, tag="crfa8")
                nc.vector.tensor_copy(a8, pt[:BL, :15])
                mx = stagep.tile([BL, 1], F32, tag="crfmx")
                nc.vector.tensor_reduce(mx, a8, axis=AX.X, op=OP.max)
                lmx = stagep.tile([BL, 1], F32, tag="crflmx")
                nc.scalar.activation(lmx, mx, AF.Ln)
                rcp = stagep.tile([BL, 1], F32, tag="crfrcp")
                nc.vector.reciprocal(rcp, mx)
                # transpose [BL,1] -> [1,BL] twice (scale and log-offset)
                ptm = ps_small.tile([15, 64 * BL], F32, tag="small")
                nc.tensor.transpose(ptm[:1, :BL], rcp, ident[:BL, :BL])
                nc.tensor.transpose(ptm[:1, BL : 2 * BL], lmx, ident[:BL, :BL])
                sc = stagep.tile([1, 2 * BL], F32, tag="crfsc")
                nc.vector.tensor_copy(sc, ptm[:1, 0 : 2 * BL])
                nc.vector.tensor_add(off_r, off_r, sc[:, BL : 2 * BL])
                scb = ps_small.tile([15, 64 * BL], F32, tag="small")
                nc.tensor.matmul(scb[:15, 0:BL], ones115, sc[:, 0:BL], start=True, stop=True)
                pn = stagep.tile([15, BL], F32, tag="pnew", bufs=4)
                nc.vector.tensor_mul(pn, pcur[0], scb[:15, 0:BL])
                pcur[0] = pn

            # python-unrolled scan: each step is only 2 tiny instructions
            for s in range(1, T):
                crf_step(s)
                if s % 64 == 0:
                    crf_renorm()

            # ---- denominator + output -------------------------------------
            pse = ps_small.tile([15, 64 * BL], F32, tag="small")
            nc.tensor.matmul(pse[:1, 0:BL], ones15, pcur[0], start=True, stop=True)
            den8 = stagep.tile([1, BL], F32, tag="den8")
            nc.scalar.activation(den8, pse[:1, 0:BL], AF.Ln)
            nc.vector.tensor_add(den8, den8, off_r)
            dsum = stagep.tile([1, 1], F32, tag="dsum")
            nc.vector.tensor_reduce(dsum, den8, axis=AX.X, op=OP.add)
            res = work.tile([1, 1], F32, tag="res", bufs=1)
            # den total = dsum + BL*511*ln16 ; res = den - num
            logc = consts.tile([1, 1], F32, tag="logc")
            nc.vector.memset(logc, float(BL * 511) * LOG16)
            nc.scalar.activation(res, dsum, AF.Identity, bias=logc)
            nc.vector.tensor_sub(res, res, num11)
            nc.sync.dma_start(out=out_ext[:, :], in_=res)

    nc.finalize()
    _BUILD_CACHE["nc"] = nc
    return nc


# ---- host-side input prep ---------------------------------------------------

_GPERM = np.concatenate([np.arange(0, 512), np.arange(768, 1024), np.arange(512, 768)])


def _wih_prep(W, dk_n):
    # lhsT tiles: [p, dk, m, c] = W[gperm[m*128+c], dk*128+p]
    Wp = W[_GPERM]
    return np.ascontiguousarray(
        Wp.reshape(8, 128, dk_n, 128).transpose(3, 2, 0, 1).reshape(128, dk_n * 8 * 128)
    ).astype(np.float16)


def _common_inputs(inputs):
    out = {}
    bias_cols = {}
    for key in ("c0", "c1", "w0", "w1"):
        Wih = np.asarray(inputs[f"{key}_Wih"], np.float32)
        Whh = np.asarray(inputs[f"{key}_Whh"], np.float32)
        bih = np.asarray(inputs[f"{key}_bih"], np.float32)
        bhh = np.asarray(inputs[f"{key}_bhh"], np.float32)
        dk_n = Wih.shape[2] // 128
        for r, sfx in ((0, "f"), (1, "b")):
            out[f"wih_{key}{sfx}"] = _wih_prep(Wih[r], dk_n)
            out[f"whh_{key}{sfx}"] = _wih_prep(Whh[r], 2)
            bb = (bih[r] + bhh[r])[_GPERM]
            bias_cols[f"{key}{sfx}"] = bb.reshape(8, 128).T  # (128, 8)
    out["biasall"] = np.ascontiguousarray(
        np.stack([bias_cols[nm] for nm in DIR_ORDER], axis=1)
    ).astype(np.float32)
    w1 = np.asarray(inputs["cls_w1"], np.float32)  # (512, 1024)
    out["cls1"] = np.ascontiguousarray(
        w1.reshape(4, 128, 8, 128).transpose(3, 2, 0, 1).reshape(128, 8 * 4 * 128)
    ).astype(np.float16)
    out["clsb1"] = np.ascontiguousarray(
        np.asarray(inputs["cls_b1"], np.float32).reshape(4, 128).T
    ).astype(np.float32)
    w2 = np.asarray(inputs["cls_w2"], np.float32)  # (15, 512)
    out["cls2"] = np.ascontiguousarray(
        w2.reshape(15, 4, 128).transpose(2, 1, 0).reshape(128, 4 * 15)
    ).astype(np.float16)
    out["clsb2"] = np.asarray(inputs["cls_b2"], np.float32).reshape(15, 1).copy()
    out["trans"] = np.asarray(inputs["crf_trans"], np.float32).copy()
    out["crfstart"] = np.asarray(inputs["crf_start"], np.float32).reshape(15, 1).copy()
    out["crfend"] = np.asarray(inputs["crf_end"], np.float32).reshape(15, 1).copy()
    return out


def make_in_maps(inputs):
    common = _common_inputs(inputs)
    char_ids = np.asarray(inputs["char_ids"])
    tags = np.asarray(inputs["tags"])
    wemb = np.asarray(inputs["word_embeddings"], np.float32)
    emb = np.asarray(inputs["char_emb_table"], np.float32)
    in_maps = []
    for c in range(NC_N):
        lo, hi = c * BL, (c + 1) * BL
        m = dict(common)
        ce = emb[char_ids[lo:hi]]  # (BL, T, 128)
        m["ceT"] = np.ascontiguousarray(
            ce.transpose(2, 1, 0).reshape(128, 1, TB)
        ).astype(np.float16)
        m["weT"] = np.ascontiguousarray(
            wemb[lo:hi].reshape(BL, T, 6, 128).transpose(3, 2, 1, 0).reshape(128, 6, TB)
        ).astype(np.float16)
        oh = (np.arange(K)[:, None, None] == tags[lo:hi][None]).astype(np.float32)
        m["tagoneT"] = np.ascontiguousarray(oh.transpose(0, 2, 1).reshape(K, TB)).astype(np.float16)
        in_maps.append(m)
    return in_maps


def kernel(**inputs):
    nc = _build_nc()
    in_maps = make_in_maps(inputs)
    res = run_bass_kernel_spmd(nc, in_maps, core_ids=list(range(NC_N)))
    total = sum(float(res.results[c]["out"][0, 0]) for c in range(NC_N))
    return np.float32(total / B)


# revision 3
# speedup vs baseline: 1.2939x; 1.0002x over previous
"""BiLSTM dual-pathway + CRF NLL kernel for 8 Trainium2 NeuronCores (V2).

Sharding: data-parallel over batch (B=64 -> 8 per core). Each core runs the
full network on its batch shard and emits a partial sum of (denom - num) over
its 8 sequences; host sums and divides by 64.

V3 structure: the 8 LSTM directions run as 2 waves of 4 concurrent
recurrences (wave1: c0f,c0b,w0f,w0b; wave2: c1f,c1b,w1f,w1b), interleaved
step-by-step inside one hardware time loop so the PE array stays warm and the
vector/scalar gate chain hides under the next directions' matmuls. The CRF
forward scan runs in linear space (p' = E_t o (M^T p), M = exp(trans)/16)
with renormalization every 64 steps, which is 2 small ops per step.

On-chip layout is feature-major: features on SBUF partitions, (t*BL + b) on
the free axis. LSTM gate order is permuted to [i, f, o, g]. Matmul operands
are fp16; state/CRF fp32. Backward-direction time reversal uses
negative-stride access patterns.
"""

import sys

sys.path.insert(0, "/opt/trn_rl_repo")

import numpy as np

import concourse.bass as bass
import concourse.mybir as mybir
from concourse import bacc
from concourse.bass import ds
from concourse.masks import make_identity
from concourse.tile import TileContext
from concourse.bass_utils import run_bass_kernel_spmd

F16 = mybir.dt.float16
F32 = mybir.dt.float32
AF = mybir.ActivationFunctionType
AX = mybir.AxisListType
OP = mybir.AluOpType

B, T, V, K = 64, 512, 40, 15
NC_N = 8
BL = B // NC_N          # 8 sequences per core
TB = T * BL             # 4096 free columns
UNROLL = 16
LOG16 = float(np.log(16.0))

# waves of 4 directions: (name, Dk chunks, source kind, reverse)
WAVE1 = [("c0f", 1, "ce", False), ("c0b", 1, "ce", True),
         ("w0f", 6, "we", False), ("w0b", 6, "we", True)]
WAVE2 = [("c1f", 4, "c0", False), ("c1b", 4, "c0", True),
         ("w1f", 4, "w0", False), ("w1b", 4, "w0", True)]
DIR_ORDER = [nm for nm, _, _, _ in WAVE1 + WAVE2]

_BUILD_CACHE = {}


def _seq_ap(tile, k, col_lo, n_steps, reverse):
    """AP over per-dir tile[:, k, :]: n_steps blocks of BL cols, fwd or rev."""
    p_step = tile.ap[0][0]
    Wd = tile.ap[2][1]          # cols per k row
    off = tile.offset + k * Wd + col_lo
    step = -BL if reverse else BL
    return bass.AP(tensor=tile.tensor, offset=off,
                   ap=[[p_step, 128], [step, n_steps], [1, BL]])


def _ps_chain_ap(ps, d):
    """[128, p(2), t(4), b(8)] view of a group psum tile for direction d."""
    p_step = ps.ap[0][0]
    return bass.AP(tensor=ps.tensor, offset=ps.offset + d * 8,
                   ap=[[p_step, 128], [512, 2], [16, 4], [1, 8]])


def _xg_chain_ap(xgt, j):
    """[128, p(2), t(4), b(8)] view of xg stage tile [128, 8, U*BL] at step j."""
    p_step = xgt.ap[0][0]
    mstride = xgt.ap[1][0]      # U*BL
    return bass.AP(tensor=xgt.tensor, offset=xgt.offset + j * BL,
                   ap=[[p_step, 128], [mstride, 2], [2 * mstride, 4], [1, 8]])


def _build_nc():
    if "nc" in _BUILD_CACHE:
        return _BUILD_CACHE["nc"]
    nc = bacc.Bacc(target_bir_lowering=False)

    # ---- external parameters -------------------------------------------------
    ceT_ext = nc.declare_dram_parameter("ceT", [128, 1, TB], F16, isOutput=False)
    weT_ext = nc.declare_dram_parameter("weT", [128, 6, TB], F16, isOutput=False)
    wih_ext, whh_ext = {}, {}
    for nm, dk, _, _ in WAVE1 + WAVE2:
        wih_ext[nm] = nc.declare_dram_parameter(f"wih_{nm}", [128, dk * 8 * 128], F16, isOutput=False)
        whh_ext[nm] = nc.declare_dram_parameter(f"whh_{nm}", [128, 2 * 8 * 128], F16, isOutput=False)
    biasall_ext = nc.declare_dram_parameter("biasall", [128, 8, 8], F32, isOutput=False)
    cls1_ext = nc.declare_dram_parameter("cls1", [128, 8 * 4 * 128], F16, isOutput=False)
    clsb1_ext = nc.declare_dram_parameter("clsb1", [128, 4], F32, isOutput=False)
    cls2_ext = nc.declare_dram_parameter("cls2", [128, 4 * 15], F16, isOutput=False)
    clsb2_ext = nc.declare_dram_parameter("clsb2", [15, 1], F32, isOutput=False)
    trans_ext = nc.declare_dram_parameter("trans", [15, 15], F32, isOutput=False)
    start_ext = nc.declare_dram_parameter("crfstart", [15, 1], F32, isOutput=False)
    end_ext = nc.declare_dram_parameter("crfend", [15, 1], F32, isOutput=False)
    tago_ext = nc.declare_dram_parameter("tagoneT", [15, TB], F16, isOutput=False)
    out_ext = nc.declare_dram_parameter("out", [1, 1], F32, isOutput=True)

    # internal DRAM: pre-activation gate inputs, one slot per wave position
    xg_dram = [nc.dram_tensor(f"xg_{i}", [128, 8, TB], F16) for i in range(4)]

    with TileContext(nc) as tc:
        with (
            tc.tile_pool(name="consts", bufs=1) as consts,
            tc.tile_pool(name="seqs", bufs=1) as seqs,
            tc.tile_pool(name="wpool", bufs=1) as wpool,
            tc.tile_pool(name="work", bufs=2) as work,
            tc.tile_pool(name="stage", bufs=3) as stagep,
            tc.tile_pool(name="ps_big", bufs=2, space="PSUM") as ps_big,
            tc.tile_pool(name="ps_rec", bufs=2, space="PSUM") as ps_rec,
            tc.tile_pool(name="ps_small", bufs=2, space="PSUM") as ps_small,
        ):
            ident = consts.tile([128, 128], F32, tag="ident")
            make_identity(nc, ident)

            ceT = consts.tile([128, 1, TB], F16, tag="ceT")
            nc.sync.dma_start(out=ceT, in_=ceT_ext[:, :, :])
            cls1 = consts.tile([128, 8, 4, 128], F16, tag="cls1")
            nc.sync.dma_start(out=cls1, in_=cls1_ext.ap().rearrange("p (k m c) -> p k m c", k=8, m=4))
            clsb1 = consts.tile([128, 4], F32, tag="clsb1")
            nc.sync.dma_start(out=clsb1, in_=clsb1_ext[:, :])
            cls2 = consts.tile([128, 4, 15], F16, tag="cls2")
            nc.sync.dma_start(out=cls2, in_=cls2_ext.ap().rearrange("p (k j) -> p k j", k=4))
            clsb2 = consts.tile([15, 1], F32, tag="clsb2")
            nc.sync.dma_start(out=clsb2, in_=clsb2_ext[:, :])
            trans = consts.tile([15, 15], F32, tag="trans")
            nc.sync.dma_start(out=trans, in_=trans_ext[:, :])
            crfstart = consts.tile([15, 1], F32, tag="crfstart")
            nc.sync.dma_start(out=crfstart, in_=start_ext[:, :])
            crfend = consts.tile([15, 1], F32, tag="crfend")
            nc.sync.dma_start(out=crfend, in_=end_ext[:, :])
            tago = consts.tile([15, TB], F16, tag="tago")
            nc.sync.dma_start(out=tago, in_=tago_ext[:, :])
            biasall = consts.tile([128, 8, 8], F32, tag="biasall")
            nc.sync.dma_start(out=biasall, in_=biasall_ext[:, :, :])

            # per-direction h-sequence buffers: [128, k(2), BL + T*BL] fp16;
            # col (s+1)*BL holds h_s in scan order (b-dirs stored reversed).
            # wave-1 buffers are reused by the matching wave-2 direction.
            hs = {}
            for slot in ("cf", "cb", "wf", "wb"):
                hs[slot] = seqs.tile([128, 2, BL + TB], F16, tag=f"hs_{slot}",
                                     name=f"hs_{slot}")
            slot_of = {nm: nm[0] + nm[2] for nm in DIR_ORDER}

            def xg_rhs_ap(src_tile, k, ns, reverse, width_steps, col_base):
                if not reverse:
                    return _seq_ap(src_tile, k, col_base + ns * 64 * BL, 64, False)
                top = width_steps - 1 - ns * 64
                return _seq_ap(src_tile, k, col_base + top * BL, 64, True)

            def xg_phase(slot, di, nm, dk_n, src_kind, reverse):
                """Gate-input matmuls for one direction -> xg_dram[slot]."""
                wih = wpool.tile([128, 6, 8, 128], F16, tag="wih", bufs=2)
                nc.sync.dma_start(
                    out=wih[:, :dk_n],
                    in_=wih_ext[nm].ap().rearrange("p (k m c) -> p k m c", k=dk_n, m=8),
                )
                for ns in range(8):
                    if src_kind == "we":
                        wxs = work.tile([128, 6, 64 * BL], F16, tag="wxs", bufs=2)
                        blk = (7 - ns) if reverse else ns
                        nc.sync.dma_start(out=wxs, in_=weT_ext[:, :, ds(blk * 64 * BL, 64 * BL)])
                    for m in range(8):
                        ps = ps_big.tile([128, 64, BL], F32, tag="xgps")
                        for dk in range(dk_n):
                            if src_kind == "ce":
                                # ceT is [128, 1, TB]; treat as k=dk? only dk=0
                                p_step = ceT.ap[0][0]
                                off = ceT.offset + (0 if not reverse else 0)
                                if not reverse:
                                    rhs = bass.AP(tensor=ceT.tensor,
                                                  offset=ceT.offset + ns * 64 * BL,
                                                  ap=[[p_step, 128], [BL, 64], [1, BL]])
                                else:
                                    top = T - 1 - ns * 64
                                    rhs = bass.AP(tensor=ceT.tensor,
                                                  offset=ceT.offset + top * BL,
                                                  ap=[[p_step, 128], [-BL, 64], [1, BL]])
                            elif src_kind == "we":
                                p_step = wxs.ap[0][0]
                                W = wxs.ap[2][1]
                                if not reverse:
                                    rhs = bass.AP(tensor=wxs.tensor,
                                                  offset=wxs.offset + dk * W,
                                                  ap=[[p_step, 128], [BL, 64], [1, BL]])
                                else:
                                    rhs = bass.AP(tensor=wxs.tensor,
                                                  offset=wxs.offset + dk * W + 63 * BL,
                                                  ap=[[p_step, 128], [-BL, 64], [1, BL]])
                            else:
                                # layer-1 input: concat(fwd h, bwd h) of layer 0
                                pre = "c" if src_kind == "c0" else "w"
                                d_src = dk // 2          # 0 = fwd dir, 1 = bwd dir
                                base = hs[pre + ("f" if d_src == 0 else "b")]
                                k_src = dk % 2
                                krev = reverse if d_src == 0 else (not reverse)
                                rhs = xg_rhs_ap(base, k_src, ns, krev, T, BL)
                            nc.tensor.matmul(ps, wih[:, dk, m], rhs,
                                             start=(dk == 0), stop=(dk == dk_n - 1))
                        st = stagep.tile([128, 64 * BL], F16, tag="xgstage")
                        nc.vector.tensor_scalar_add(st, ps, biasall[:, di, m : m + 1])
                        nc.sync.dma_start(out=xg_dram[slot][:, m, ds(ns * 64 * BL, 64 * BL)], in_=st)

            def rec_wave(wave, whh_tiles):
                """One wave: 4 directions (2 groups of f/b) interleaved in time.

                Per group one psum tile [128, 2, 512] spanning 2 banks; gate
                tile m goes to bank m%2 at col (m//2)*16 + d*8, so consecutive
                matmul accumulation groups alternate banks (overlapping one
                matmul's drain with the next one's fill).
                """
                hts = [[hs[slot_of[wave[2 * gi + d][0]]] for d in range(2)]
                       for gi in range(2)]
                csts = []
                for gi in range(2):
                    dirc = []
                    for d in range(2):
                        cst = work.tile([128, 2, BL], F32, tag=f"cst{gi}{d}", bufs=1,
                                        name=f"cst{gi}{d}")
                        nc.vector.memset(cst, 0.0)
                        nc.vector.memset(hts[gi][d][:, :, 0:BL], 0.0)
                        dirc.append(cst)
                    csts.append(dirc)
                with tc.For_i(0, T, UNROLL) as tv:
                    xgs = []
                    for di4 in range(4):
                        xgt = stagep.tile([128, 8, UNROLL * BL], F16, tag=f"xgs{di4}",
                                          name=f"xgs{di4}", bufs=2)
                        nc.sync.dma_start(out=xgt, in_=xg_dram[di4][:, :, ds(tv * BL, UNROLL * BL)])
                        xgs.append(xgt)
                    for j in range(UNROLL):
                        for gi in (range(2) if j % 2 == 0 else (1, 0)):
                            whh = whh_tiles[gi]
                            # one bank per buf; col = m*16 + d*8 + b
                            ps = ps_rec.tile([128, 512], F32, tag=f"recps{gi}",
                                             name=f"recps{gi}")
                            for d in range(2):
                                hst = hts[gi][d]
                                for m in range(8):
                                    out = ps[:, ds(m * 16 + d * 8, BL)]
                                    for k in range(2):
                                        nc.tensor.matmul(
                                            out, whh[:, d, k, m],
                                            hst[:, k, ds(tv * BL + j * BL, BL)],
                                            start=(k == 0), stop=(k == 1),
                                        )
                            for d in range(2):
                                hst, cst = hts[gi][d], csts[gi][d]
                                p_step = ps.ap[0][0]
                                ps_view = bass.AP(tensor=ps.tensor,
                                                  offset=ps.offset + d * BL,
                                                  ap=[[p_step, 128], [16, 8], [1, BL]])
                                g = stagep.tile([128, 8, BL], F32, tag=f"g{gi}{d}",
                                                name=f"g{gi}{d}")
                                nc.vector.tensor_add(g, ps_view,
                                                     xgs[2 * gi + d][:, :, ds(j * BL, BL)])
                                sig = stagep.tile([128, 6, BL], F32, tag=f"sig{gi}{d}",
                                                  name=f"sig{gi}{d}")
                                nc.scalar.activation(sig, g[:, 0:6], AF.Sigmoid)
                                tgg = stagep.tile([128, 2, BL], F32, tag=f"tgg{gi}{d}",
                                                  name=f"tgg{gi}{d}")
                                nc.scalar.activation(tgg, g[:, 6:8], AF.Tanh)
                                tmp = stagep.tile([128, 2, BL], F32, tag=f"tmp{gi}{d}",
                                                  name=f"tmp{gi}{d}")
                                nc.vector.tensor_mul(tmp, sig[:, 0:2], tgg)
                                nc.vector.tensor_mul(cst, cst, sig[:, 2:4])
                                nc.vector.tensor_add(cst, cst, tmp)
                                tch = stagep.tile([128, 2, BL], F32, tag=f"tch{gi}{d}",
                                                  name=f"tch{gi}{d}")
                                nc.scalar.activation(tch, cst, AF.Tanh)
                                nc.vector.tensor_mul(
                                    hst[:, :, ds(tv * BL + j * BL + BL, BL)],
                                    sig[:, 4:6], tch)

            def load_whh(wave):
                tiles = []
                for gi in range(2):
                    whh = wpool.tile([128, 2, 2, 8, 128], F16, tag=f"whh{gi}",
                                     name=f"whh{gi}")
                    for d in range(2):
                        nm = wave[2 * gi + d][0]
                        nc.sync.dma_start(
                            out=whh[:, d],
                            in_=whh_ext[nm].ap().rearrange("p (k m c) -> p k m c", k=2, m=8))
                    tiles.append(whh)
                return tiles

            # ---- wave 1 ----------------------------------------------------
            for slot, (nm, dk_n, src, rev) in enumerate(WAVE1):
                xg_phase(slot, DIR_ORDER.index(nm), nm, dk_n, src, rev)
            rec_wave(WAVE1, load_whh(WAVE1))

            # ---- wave 2 ----------------------------------------------------
            for slot, (nm, dk_n, src, rev) in enumerate(WAVE2):
                xg_phase(slot, DIR_ORDER.index(nm), nm, dk_n, src, rev)
            rec_wave(WAVE2, load_whh(WAVE2))

            # ---- classifier + CRF inputs ----------------------------------
            # comb chunk kk -> (group, d, k): [c1f k0,k1, c1b k0,k1, w1f.., w1b..]
            Et = seqs.tile([15, TB], F16, tag="Et")            # exp(logits)

            racc = work.tile([15, 16], F32, tag="racc", bufs=1)
            nc.vector.memset(racc, 0.0)
            trans16 = consts.tile([15, 15], F16, tag="trans16")
            nc.vector.tensor_copy(trans16, trans)

            def comb_rhs(kk, ns):
                names = ["cf", "cb", "wf", "wb"]
                base = hs[names[kk // 2]]
                rev = (kk // 2) % 2 == 1
                return xg_rhs_ap(base, kk % 2, ns, rev, T, BL)

            for ns in range(8):
                hmt = []
                for m in range(4):
                    ps = ps_big.tile([128, 64, BL], F32, tag="xgps")
                    for kk in range(8):
                        nc.tensor.matmul(ps, cls1[:, kk, m], comb_rhs(kk, ns),
                                         start=(kk == 0), stop=(kk == 7))
                    hm = stagep.tile([128, 64 * BL], F16, tag="hm", bufs=4, name=f"hm{m}")
                    nc.scalar.activation(hm, ps, AF.Relu, bias=clsb1[:, m : m + 1])
                    hmt.append(hm)
                ps2 = ps_small.tile([15, 64 * BL], F32, tag="small")
                for m in range(4):
                    nc.tensor.matmul(ps2, cls2[:, m], hmt[m], start=(m == 0), stop=(m == 3))
                lg = stagep.tile([15, 64 * BL], F32, tag="lgits")
                nc.vector.tensor_scalar_add(lg, ps2, clsb2)
                if ns == 0:
                    nc.vector.tensor_scalar_add(lg[:, 0:BL], lg[:, 0:BL], crfstart)
                if ns == 7:
                    nc.vector.tensor_scalar_add(lg[:, 64 * BL - BL :], lg[:, 64 * BL - BL :], crfend)
                nc.scalar.activation(Et[:, ds(ns * 64 * BL, 64 * BL)], lg, AF.Exp)

                # CRF numerator pieces on this tile
                psv = ps_small.tile([15, 64 * BL], F32, tag="small")
                nc.tensor.matmul(psv, trans16, tago[:, ds(ns * 64 * BL, 64 * BL)], start=True, stop=True)
                w = 64 * BL if ns < 7 else 64 * BL - BL
                pr = stagep.tile([15, 64 * BL], F32, tag="prodns")
                nc.vector.tensor_mul(pr[:, :w], psv[:, :w], tago[:, ds(ns * 64 * BL + BL, w)])
                nc.vector.tensor_reduce(racc[:, ns : ns + 1], pr[:, :w],
                                        axis=AX.X, op=OP.add)
                pr2 = stagep.tile([15, 64 * BL], F32, tag="prodns")
                nc.vector.tensor_mul(pr2, lg, tago[:, ds(ns * 64 * BL, 64 * BL)])
                nc.vector.tensor_reduce(racc[:, 8 + ns : 9 + ns], pr2,
                                        axis=AX.X, op=OP.add)

            nv = stagep.tile([15, 1], F32, tag="nv")
            nc.vector.tensor_reduce(nv, racc, axis=AX.X, op=OP.add)
            ones15 = consts.tile([15, 1], F32, tag="ones15")
            nc.vector.memset(ones15, 1.0)
            psn = ps_small.tile([15, 64 * BL], F32, tag="small")
            nc.tensor.matmul(psn[:1, :1], ones15, nv, start=True, stop=True)
            num11 = work.tile([1, 1], F32, tag="num11", bufs=1)
            nc.vector.tensor_copy(num11, psn[:1, :1])

            # ---- CRF forward scan, linear space ---------------------------
            # p_t = E_t o (Mexp^T p_{t-1}), Mexp = exp(trans)/16 (fp32 matmul).
            # logZ = ln(sum_j p_T) + 511*ln16 + renorm offsets.
            mexp = consts.tile([15, 15], F32, tag="mexp")
            nlog16 = consts.tile([15, 1], F32, tag="nlog16")
            nc.vector.memset(nlog16, -LOG16)
            nc.scalar.activation(mexp, trans, AF.Exp, bias=nlog16)
            ones115 = consts.tile([1, 15], F32, tag="ones115")
            nc.vector.memset(ones115, 1.0)

            # start/end potentials are already folded into logits (hence Et)
            p0 = seqs.tile([15, BL], F32, tag="p0")
            nc.vector.tensor_copy(p0, Et[:, 0:BL])
            off_r = seqs.tile([1, BL], F32, tag="off_r")
            nc.vector.memset(off_r, 0.0)

            pcur = [p0]

            def crf_step(s):
                z = ps_small.tile([15, 64 * BL], F32, tag="small")
                nc.tensor.matmul(z[:, 0:BL], mexp, pcur[0], start=True, stop=True)
                pn = stagep.tile([15, BL], F32, tag="pnew", bufs=4)
                nc.vector.tensor_mul(pn, z[:, 0:BL], Et[:, ds(s * BL, BL)])
                pcur[0] = pn

            def crf_renorm():
                # per-batch max over states -> scale p to ~1, accumulate log
                pt = ps_small.tile([15, 64 * BL], F32, tag="small")
                nc.tensor.transpose(pt[:BL, :15], pcur[0], ident[:15, :15])
                a8 = stagep.tile([BL, 15], F32, tag="crfa8")
                nc.vector.tensor_copy(a8, pt[:BL, :15])
                mx = stagep.tile([BL, 1], F32, tag="crfmx")
                nc.vector.tensor_reduce(mx, a8, axis=AX.X, op=OP.max)
                lmx = stagep.tile([BL, 1], F32, tag="crflmx")
                nc.scalar.activation(lmx, mx, AF.Ln)
                rcp = stagep.tile([BL, 1], F32, tag="crfrcp")
                nc.vector.reciprocal(rcp, mx)
                # transpose [BL,1] -> [1,BL] twice (scale and log-offset)
                ptm = ps_small.tile([15, 64 * BL], F32, tag="small")
                nc.tensor.transpose(ptm[:1, :BL], rcp, ident[:BL, :BL])
                nc.tensor.transpose(ptm[:1, BL : 2 * BL], lmx, ident[:BL, :BL])
                sc = stagep.tile([1, 2 * BL], F32, tag="crfsc")
                nc.vector.tensor_copy(sc, ptm[:1, 0 : 2 * BL])
                nc.vector.tensor_add(off_r, off_r, sc[:, BL : 2 * BL])
                scb = ps_small.tile([15, 64 * BL], F32, tag="small")
                nc.tensor.matmul(scb[:15, 0:BL], ones115, sc[:, 0:BL], start=True, stop=True)
                pn = stagep.tile([15, BL], F32, tag="pnew", bufs=4)
                nc.vector.tensor_mul(pn, pcur[0], scb[:15, 0:BL])
                pcur[0] = pn

            # python-unrolled scan: each step is only 2 tiny instructions
            for s in range(1, T):
                crf_step(s)
                if s % 64 == 0:
                    crf_renorm()

            # ---- denominator + output -------------------------------------
            pse = ps_small.tile([15, 64 * BL], F32, tag="small")
            nc.tensor.matmul(pse[:1, 0:BL], ones15, pcur[0], start=True, stop=True)
            den8 = stagep.tile([1, BL], F32, tag="den8")
            nc.scalar.activation(den8, pse[:1, 0:BL], AF.Ln)
            nc.vector.tensor_add(den8, den8, off_r)
            dsum = stagep.tile([1, 1], F32, tag="dsum")
            nc.vector.tensor_reduce(dsum, den8, axis=AX.X, op=OP.add)
            res = work.tile([1, 1], F32, tag="res", bufs=1)
            # den total = dsum + BL*511*ln16 ; res = den - num
            logc = consts.tile([1, 1], F32, tag="logc")
            nc.vector.memset(logc, float(BL * 511) * LOG16)
            nc.scalar.activation(res, dsum, AF.Identity, bias=logc)
            nc.vector.tensor_sub(res, res, num11)
            nc.sync.dma_start(out=out_ext[:, :], in_=res)

    nc.finalize()
    _BUILD_CACHE["nc"] = nc
    return nc


# ---- host-side input prep ---------------------------------------------------

_GPERM = np.concatenate([np.arange(0, 512), np.arange(768, 1024), np.arange(512, 768)])


def _wih_prep(W, dk_n):
    # lhsT tiles: [p, dk, m, c] = W[gperm[m*128+c], dk*128+p]
    Wp = W[_GPERM]
    return np.ascontiguousarray(
        Wp.reshape(8, 128, dk_n, 128).transpose(3, 2, 0, 1).reshape(128, dk_n * 8 * 128)
    ).astype(np.float16)


def _common_inputs(inputs):
    out = {}
    bias_cols = {}
    for key in ("c0", "c1", "w0", "w1"):
        Wih = np.asarray(inputs[f"{key}_Wih"], np.float32)
        Whh = np.asarray(inputs[f"{key}_Whh"], np.float32)
        bih = np.asarray(inputs[f"{key}_bih"], np.float32)
        bhh = np.asarray(inputs[f"{key}_bhh"], np.float32)
        dk_n = Wih.shape[2] // 128
        for r, sfx in ((0, "f"), (1, "b")):
            out[f"wih_{key}{sfx}"] = _wih_prep(Wih[r], dk_n)
            out[f"whh_{key}{sfx}"] = _wih_prep(Whh[r], 2)
            bb = (bih[r] + bhh[r])[_GPERM]
            bias_cols[f"{key}{sfx}"] = bb.reshape(8, 128).T  # (128, 8)
    out["biasall"] = np.ascontiguousarray(
        np.stack([bias_cols[nm] for nm in DIR_ORDER], axis=1)
    ).astype(np.float32)
    w1 = np.asarray(inputs["cls_w1"], np.float32)  # (512, 1024)
    out["cls1"] = np.ascontiguousarray(
        w1.reshape(4, 128, 8, 128).transpose(3, 2, 0, 1).reshape(128, 8 * 4 * 128)
    ).astype(np.float16)
    out["clsb1"] = np.ascontiguousarray(
        np.asarray(inputs["cls_b1"], np.float32).reshape(4, 128).T
    ).astype(np.float32)
    w2 = np.asarray(inputs["cls_w2"], np.float32)  # (15, 512)
    out["cls2"] = np.ascontiguousarray(
        w2.reshape(15, 4, 128).transpose(2, 1, 0).reshape(128, 4 * 15)
    ).astype(np.float16)
    out["clsb2"] = np.asarray(inputs["cls_b2"], np.float32).reshape(15, 1).copy()
    out["trans"] = np.asarray(inputs["crf_trans"], np.float32).copy()
    out["crfstart"] = np.asarray(inputs["crf_start"], np.float32).reshape(15, 1).copy()
    out["crfend"] = np.asarray(inputs["crf_end"], np.float32).reshape(15, 1).copy()
    return out


def make_in_maps(inputs):
    common = _common_inputs(inputs)
    char_ids = np.asarray(inputs["char_ids"])
    tags = np.asarray(inputs["tags"])
    wemb = np.asarray(inputs["word_embeddings"], np.float32)
    emb = np.asarray(inputs["char_emb_table"], np.float32)
    in_maps = []
    for c in range(NC_N):
        lo, hi = c * BL, (c + 1) * BL
        m = dict(common)
        ce = emb[char_ids[lo:hi]]  # (BL, T, 128)
        m["ceT"] = np.ascontiguousarray(
            ce.transpose(2, 1, 0).reshape(128, 1, TB)
        ).astype(np.float16)
        m["weT"] = np.ascontiguousarray(
            wemb[lo:hi].reshape(BL, T, 6, 128).transpose(3, 2, 1, 0).reshape(128, 6, TB)
        ).astype(np.float16)
        oh = (np.arange(K)[:, None, None] == tags[lo:hi][None]).astype(np.float32)
        m["tagoneT"] = np.ascontiguousarray(oh.transpose(0, 2, 1).reshape(K, TB)).astype(np.float16)
        in_maps.append(m)
    return in_maps


def kernel(**inputs):
    nc = _build_nc()
    in_maps = make_in_maps(inputs)
    res = run_bass_kernel_spmd(nc, in_maps, core_ids=list(range(NC_N)))
    total = sum(float(res.results[c]["out"][0, 0]) for c in range(NC_N))
    return np.float32(total / B)


# revision 4
# speedup vs baseline: 1.2946x; 1.0005x over previous
"""BiLSTM dual-pathway + CRF NLL kernel for 8 Trainium2 NeuronCores (V2).

Sharding: data-parallel over batch (B=64 -> 8 per core). Each core runs the
full network on its batch shard and emits a partial sum of (denom - num) over
its 8 sequences; host sums and divides by 64.

V3 structure: the 8 LSTM directions run as 2 waves of 4 concurrent
recurrences (wave1: c0f,c0b,w0f,w0b; wave2: c1f,c1b,w1f,w1b), interleaved
step-by-step inside one hardware time loop so the PE array stays warm and the
vector/scalar gate chain hides under the next directions' matmuls. The CRF
forward scan runs in linear space (p' = E_t o (M^T p), M = exp(trans)/16)
with renormalization every 64 steps, which is 2 small ops per step.

On-chip layout is feature-major: features on SBUF partitions, (t*BL + b) on
the free axis. LSTM gate order is permuted to [i, f, o, g]. Matmul operands
are fp16; state/CRF fp32. Backward-direction time reversal uses
negative-stride access patterns.
"""

import sys

sys.path.insert(0, "/opt/trn_rl_repo")

import numpy as np

import concourse.bass as bass
import concourse.mybir as mybir
from concourse import bacc
from concourse.bass import ds
from concourse.masks import make_identity
from concourse.tile import TileContext
from concourse.bass_utils import run_bass_kernel_spmd

F16 = mybir.dt.float16
F32 = mybir.dt.float32
AF = mybir.ActivationFunctionType
AX = mybir.AxisListType
OP = mybir.AluOpType

B, T, V, K = 64, 512, 40, 15
NC_N = 8
BL = B // NC_N          # 8 sequences per core
TB = T * BL             # 4096 free columns
UNROLL = 16
LOG16 = float(np.log(16.0))

# waves of 4 directions: (name, Dk chunks, source kind, reverse)
WAVE1 = [("c0f", 1, "ce", False), ("c0b", 1, "ce", True),
         ("w0f", 6, "we", False), ("w0b", 6, "we", True)]
WAVE2 = [("c1f", 4, "c0", False), ("c1b", 4, "c0", True),
         ("w1f", 4, "w0", False), ("w1b", 4, "w0", True)]
DIR_ORDER = [nm for nm, _, _, _ in WAVE1 + WAVE2]

_BUILD_CACHE = {}


def _seq_ap(tile, k, col_lo, n_steps, reverse):
    """AP over per-dir tile[:, k, :]: n_steps blocks of BL cols, fwd or rev."""
    p_step = tile.ap[0][0]
    Wd = tile.ap[2][1]          # cols per k row
    off = tile.offset + k * Wd + col_lo
    step = -BL if reverse else BL
    return bass.AP(tensor=tile.tensor, offset=off,
                   ap=[[p_step, 128], [step, n_steps], [1, BL]])


def _ps_chain_ap(ps, d):
    """[128, p(2), t(4), b(8)] view of a group psum tile for direction d."""
    p_step = ps.ap[0][0]
    return bass.AP(tensor=ps.tensor, offset=ps.offset + d * 8,
                   ap=[[p_step, 128], [512, 2], [16, 4], [1, 8]])


def _xg_chain_ap(xgt, j):
    """[128, p(2), t(4), b(8)] view of xg stage tile [128, 8, U*BL] at step j."""
    p_step = xgt.ap[0][0]
    mstride = xgt.ap[1][0]      # U*BL
    return bass.AP(tensor=xgt.tensor, offset=xgt.offset + j * BL,
                   ap=[[p_step, 128], [mstride, 2], [2 * mstride, 4], [1, 8]])


def _build_nc():
    if "nc" in _BUILD_CACHE:
        return _BUILD_CACHE["nc"]
    nc = bacc.Bacc(target_bir_lowering=False)

    # ---- external parameters -------------------------------------------------
    ceT_ext = nc.declare_dram_parameter("ceT", [128, 1, TB], F16, isOutput=False)
    weT_ext = nc.declare_dram_parameter("weT", [128, 6, TB], F16, isOutput=False)
    wih_ext, whh_ext = {}, {}
    for nm, dk, _, _ in WAVE1 + WAVE2:
        wih_ext[nm] = nc.declare_dram_parameter(f"wih_{nm}", [128, dk * 8 * 128], F16, isOutput=False)
        whh_ext[nm] = nc.declare_dram_parameter(f"whh_{nm}", [128, 2 * 8 * 128], F16, isOutput=False)
    biasall_ext = nc.declare_dram_parameter("biasall", [128, 8, 8], F32, isOutput=False)
    cls1_ext = nc.declare_dram_parameter("cls1", [128, 8 * 4 * 128], F16, isOutput=False)
    clsb1_ext = nc.declare_dram_parameter("clsb1", [128, 4], F32, isOutput=False)
    cls2_ext = nc.declare_dram_parameter("cls2", [128, 4 * 15], F16, isOutput=False)
    clsb2_ext = nc.declare_dram_parameter("clsb2", [15, 1], F32, isOutput=False)
    trans_ext = nc.declare_dram_parameter("trans", [15, 15], F32, isOutput=False)
    start_ext = nc.declare_dram_parameter("crfstart", [15, 1], F32, isOutput=False)
    end_ext = nc.declare_dram_parameter("crfend", [15, 1], F32, isOutput=False)
    tago_ext = nc.declare_dram_parameter("tagoneT", [15, TB], F16, isOutput=False)
    out_ext = nc.declare_dram_parameter("out", [1, 1], F32, isOutput=True)

    # internal DRAM: pre-activation gate inputs, one slot per wave position
    xg_dram = [nc.dram_tensor(f"xg_{i}", [128, 8, TB], F16) for i in range(4)]

    with TileContext(nc) as tc:
        with (
            tc.tile_pool(name="consts", bufs=1) as consts,
            tc.tile_pool(name="seqs", bufs=1) as seqs,
            tc.tile_pool(name="wpool", bufs=1) as wpool,
            tc.tile_pool(name="work", bufs=2) as work,
            tc.tile_pool(name="stage", bufs=3) as stagep,
            tc.tile_pool(name="ps_big", bufs=2, space="PSUM") as ps_big,
            tc.tile_pool(name="ps_rec", bufs=2, space="PSUM") as ps_rec,
            tc.tile_pool(name="ps_small", bufs=2, space="PSUM") as ps_small,
        ):
            ident = consts.tile([128, 128], F32, tag="ident")
            make_identity(nc, ident)

            ceT = consts.tile([128, 1, TB], F16, tag="ceT")
            nc.sync.dma_start(out=ceT, in_=ceT_ext[:, :, :])
            cls1 = consts.tile([128, 8, 4, 128], F16, tag="cls1")
            nc.sync.dma_start(out=cls1, in_=cls1_ext.ap().rearrange("p (k m c) -> p k m c", k=8, m=4))
            clsb1 = consts.tile([128, 4], F32, tag="clsb1")
            nc.sync.dma_start(out=clsb1, in_=clsb1_ext[:, :])
            cls2 = consts.tile([128, 4, 15], F16, tag="cls2")
            nc.sync.dma_start(out=cls2, in_=cls2_ext.ap().rearrange("p (k j) -> p k j", k=4))
            clsb2 = consts.tile([15, 1], F32, tag="clsb2")
            nc.sync.dma_start(out=clsb2, in_=clsb2_ext[:, :])
            trans = consts.tile([15, 15], F32, tag="trans")
            nc.sync.dma_start(out=trans, in_=trans_ext[:, :])
            crfstart = consts.tile([15, 1], F32, tag="crfstart")
            nc.sync.dma_start(out=crfstart, in_=start_ext[:, :])
            crfend = consts.tile([15, 1], F32, tag="crfend")
            nc.sync.dma_start(out=crfend, in_=end_ext[:, :])
            tago = consts.tile([15, TB], F16, tag="tago")
            nc.sync.dma_start(out=tago, in_=tago_ext[:, :])
            biasall = consts.tile([128, 8, 8], F32, tag="biasall")
            nc.sync.dma_start(out=biasall, in_=biasall_ext[:, :, :])

            # per-direction h-sequence buffers: [128, k(2), BL + T*BL] fp16;
            # col (s+1)*BL holds h_s in scan order (b-dirs stored reversed).
            # wave-1 buffers are reused by the matching wave-2 direction.
            hs = {}
            for slot in ("cf", "cb", "wf", "wb"):
                hs[slot] = seqs.tile([128, 2, BL + TB], F16, tag=f"hs_{slot}",
                                     name=f"hs_{slot}")
            slot_of = {nm: nm[0] + nm[2] for nm in DIR_ORDER}

            def xg_rhs_ap(src_tile, k, ns, reverse, width_steps, col_base):
                if not reverse:
                    return _seq_ap(src_tile, k, col_base + ns * 64 * BL, 64, False)
                top = width_steps - 1 - ns * 64
                return _seq_ap(src_tile, k, col_base + top * BL, 64, True)

            def xg_phase(slot, di, nm, dk_n, src_kind, reverse):
                """Gate-input matmuls for one direction -> xg_dram[slot]."""
                wih = wpool.tile([128, 6, 8, 128], F16, tag="wih", bufs=2)
                nc.sync.dma_start(
                    out=wih[:, :dk_n],
                    in_=wih_ext[nm].ap().rearrange("p (k m c) -> p k m c", k=dk_n, m=8),
                )
                for ns in range(8):
                    if src_kind == "we":
                        wxs = work.tile([128, 6, 64 * BL], F16, tag="wxs", bufs=2)
                        blk = (7 - ns) if reverse else ns
                        nc.sync.dma_start(out=wxs, in_=weT_ext[:, :, ds(blk * 64 * BL, 64 * BL)])
                    for m in range(8):
                        ps = ps_big.tile([128, 64, BL], F32, tag="xgps")
                        for dk in range(dk_n):
                            if src_kind == "ce":
                                # ceT is [128, 1, TB]; treat as k=dk? only dk=0
                                p_step = ceT.ap[0][0]
                                off = ceT.offset + (0 if not reverse else 0)
                                if not reverse:
                                    rhs = bass.AP(tensor=ceT.tensor,
                                                  offset=ceT.offset + ns * 64 * BL,
                                                  ap=[[p_step, 128], [BL, 64], [1, BL]])
                                else:
                                    top = T - 1 - ns * 64
                                    rhs = bass.AP(tensor=ceT.tensor,
                                                  offset=ceT.offset + top * BL,
                                                  ap=[[p_step, 128], [-BL, 64], [1, BL]])
                            elif src_kind == "we":
                                p_step = wxs.ap[0][0]
                                W = wxs.ap[2][1]
                                if not reverse:
                                    rhs = bass.AP(tensor=wxs.tensor,
                                                  offset=wxs.offset + dk * W,
                                                  ap=[[p_step, 128], [BL, 64], [1, BL]])
                                else:
                                    rhs = bass.AP(tensor=wxs.tensor,
                                                  offset=wxs.offset + dk * W + 63 * BL,
                                                  ap=[[p_step, 128], [-BL, 64], [1, BL]])
                            else:
                                # layer-1 input: concat(fwd h, bwd h) of layer 0
                                pre = "c" if src_kind == "c0" else "w"
                                d_src = dk // 2          # 0 = fwd dir, 1 = bwd dir
                                base = hs[pre + ("f" if d_src == 0 else "b")]
                                k_src = dk % 2
                                krev = reverse if d_src == 0 else (not reverse)
                                rhs = xg_rhs_ap(base, k_src, ns, krev, T, BL)
                            nc.tensor.matmul(ps, wih[:, dk, m], rhs,
                                             start=(dk == 0), stop=(dk == dk_n - 1))
                        st = stagep.tile([128, 64 * BL], F16, tag="xgstage")
                        nc.vector.tensor_scalar_add(st, ps, biasall[:, di, m : m + 1])
                        nc.sync.dma_start(out=xg_dram[slot][:, m, ds(ns * 64 * BL, 64 * BL)], in_=st)

            def rec_wave(wave, whh_tiles):
                """One wave: 4 directions (2 groups of f/b) interleaved in time.

                Per group one psum tile [128, 2, 512] spanning 2 banks; gate
                tile m goes to bank m%2 at col (m//2)*16 + d*8, so consecutive
                matmul accumulation groups alternate banks (overlapping one
                matmul's drain with the next one's fill).
                """
                hts = [[hs[slot_of[wave[2 * gi + d][0]]] for d in range(2)]
                       for gi in range(2)]
                csts = []
                for gi in range(2):
                    dirc = []
                    for d in range(2):
                        cst = work.tile([128, 2, BL], F32, tag=f"cst{gi}{d}", bufs=1,
                                        name=f"cst{gi}{d}")
                        nc.vector.memset(cst, 0.0)
                        nc.vector.memset(hts[gi][d][:, :, 0:BL], 0.0)
                        dirc.append(cst)
                    csts.append(dirc)
                lbl = f"recbe{id(whh_tiles) % 9973}"
                with tc.For_i(0, T, UNROLL, back_edge_label=lbl) as tv:
                    tc.mark_branch_hint_location(
                        lbl,
                        engines=[mybir.EngineType.PE, mybir.EngineType.DVE,
                                 mybir.EngineType.Activation, mybir.EngineType.SP,
                                 mybir.EngineType.Pool])
                    xgs = []
                    for di4 in range(4):
                        xgt = stagep.tile([128, 8, UNROLL * BL], F16, tag=f"xgs{di4}",
                                          name=f"xgs{di4}", bufs=2)
                        nc.sync.dma_start(out=xgt, in_=xg_dram[di4][:, :, ds(tv * BL, UNROLL * BL)])
                        xgs.append(xgt)
                    for j in range(UNROLL):
                        for gi in (range(2) if j % 2 == 0 else (1, 0)):
                            whh = whh_tiles[gi]
                            # one bank per buf; col = m*16 + d*8 + b
                            ps = ps_rec.tile([128, 512], F32, tag=f"recps{gi}",
                                             name=f"recps{gi}")
                            for d in range(2):
                                hst = hts[gi][d]
                                for m in range(8):
                                    out = ps[:, ds(m * 16 + d * 8, BL)]
                                    for k in range(2):
                                        nc.tensor.matmul(
                                            out, whh[:, d, k, m],
                                            hst[:, k, ds(tv * BL + j * BL, BL)],
                                            start=(k == 0), stop=(k == 1),
                                        )
                            for d in range(2):
                                hst, cst = hts[gi][d], csts[gi][d]
                                p_step = ps.ap[0][0]
                                ps_view = bass.AP(tensor=ps.tensor,
                                                  offset=ps.offset + d * BL,
                                                  ap=[[p_step, 128], [16, 8], [1, BL]])
                                g = stagep.tile([128, 8, BL], F32, tag=f"g{gi}{d}",
                                                name=f"g{gi}{d}")
                                nc.vector.tensor_add(g, ps_view,
                                                     xgs[2 * gi + d][:, :, ds(j * BL, BL)])
                                sig = stagep.tile([128, 6, BL], F32, tag=f"sig{gi}{d}",
                                                  name=f"sig{gi}{d}")
                                nc.scalar.activation(sig, g[:, 0:6], AF.Sigmoid)
                                tgg = stagep.tile([128, 2, BL], F32, tag=f"tgg{gi}{d}",
                                                  name=f"tgg{gi}{d}")
                                nc.scalar.activation(tgg, g[:, 6:8], AF.Tanh)
                                tmp = stagep.tile([128, 2, BL], F32, tag=f"tmp{gi}{d}",
                                                  name=f"tmp{gi}{d}")
                                nc.vector.tensor_mul(tmp, sig[:, 0:2], tgg)
                                nc.vector.tensor_mul(cst, cst, sig[:, 2:4])
                                nc.vector.tensor_add(cst, cst, tmp)
                                tch = stagep.tile([128, 2, BL], F32, tag=f"tch{gi}{d}",
                                                  name=f"tch{gi}{d}")
                                nc.scalar.activation(tch, cst, AF.Tanh)
                                nc.vector.tensor_mul(
                                    hst[:, :, ds(tv * BL + j * BL + BL, BL)],
                                    sig[:, 4:6], tch)

            def load_whh(wave):
                tiles = []
                for gi in range(2):
                    whh = wpool.tile([128, 2, 2, 8, 128], F16, tag=f"whh{gi}",
                                     name=f"whh{gi}")
                    for d in range(2):
                        nm = wave[2 * gi + d][0]
                        nc.sync.dma_start(
                            out=whh[:, d],
                            in_=whh_ext[nm].ap().rearrange("p (k m c) -> p k m c", k=2, m=8))
                    tiles.append(whh)
                return tiles

            # ---- wave 1 ----------------------------------------------------
            for slot, (nm, dk_n, src, rev) in enumerate(WAVE1):
                xg_phase(slot, DIR_ORDER.index(nm), nm, dk_n, src, rev)
            rec_wave(WAVE1, load_whh(WAVE1))

            # ---- wave 2 ----------------------------------------------------
            for slot, (nm, dk_n, src, rev) in enumerate(WAVE2):
                xg_phase(slot, DIR_ORDER.index(nm), nm, dk_n, src, rev)
            rec_wave(WAVE2, load_whh(WAVE2))

            # ---- classifier + CRF inputs ----------------------------------
            # comb chunk kk -> (group, d, k): [c1f k0,k1, c1b k0,k1, w1f.., w1b..]
            Et = seqs.tile([15, TB], F16, tag="Et")            # exp(logits)

            racc = work.tile([15, 16], F32, tag="racc", bufs=1)
            nc.vector.memset(racc, 0.0)
            trans16 = consts.tile([15, 15], F16, tag="trans16")
            nc.vector.tensor_copy(trans16, trans)

            def comb_rhs(kk, ns):
                names = ["cf", "cb", "wf", "wb"]
                base = hs[names[kk // 2]]
                rev = (kk // 2) % 2 == 1
                return xg_rhs_ap(base, kk % 2, ns, rev, T, BL)

            for ns in range(8):
                hmt = []
                for m in range(4):
                    ps = ps_big.tile([128, 64, BL], F32, tag="xgps")
                    for kk in range(8):
                        nc.tensor.matmul(ps, cls1[:, kk, m], comb_rhs(kk, ns),
                                         start=(kk == 0), stop=(kk == 7))
                    hm = stagep.tile([128, 64 * BL], F16, tag="hm", bufs=4, name=f"hm{m}")
                    nc.scalar.activation(hm, ps, AF.Relu, bias=clsb1[:, m : m + 1])
                    hmt.append(hm)
                ps2 = ps_small.tile([15, 64 * BL], F32, tag="small")
                for m in range(4):
                    nc.tensor.matmul(ps2, cls2[:, m], hmt[m], start=(m == 0), stop=(m == 3))
                lg = stagep.tile([15, 64 * BL], F32, tag="lgits")
                nc.vector.tensor_scalar_add(lg, ps2, clsb2)
                if ns == 0:
                    nc.vector.tensor_scalar_add(lg[:, 0:BL], lg[:, 0:BL], crfstart)
                if ns == 7:
                    nc.vector.tensor_scalar_add(lg[:, 64 * BL - BL :], lg[:, 64 * BL - BL :], crfend)
                nc.scalar.activation(Et[:, ds(ns * 64 * BL, 64 * BL)], lg, AF.Exp)

                # CRF numerator pieces on this tile
                psv = ps_small.tile([15, 64 * BL], F32, tag="small")
                nc.tensor.matmul(psv, trans16, tago[:, ds(ns * 64 * BL, 64 * BL)], start=True, stop=True)
                w = 64 * BL if ns < 7 else 64 * BL - BL
                pr = stagep.tile([15, 64 * BL], F32, tag="prodns")
                nc.vector.tensor_mul(pr[:, :w], psv[:, :w], tago[:, ds(ns * 64 * BL + BL, w)])
                nc.vector.tensor_reduce(racc[:, ns : ns + 1], pr[:, :w],
                                        axis=AX.X, op=OP.add)
                pr2 = stagep.tile([15, 64 * BL], F32, tag="prodns")
                nc.vector.tensor_mul(pr2, lg, tago[:, ds(ns * 64 * BL, 64 * BL)])
                nc.vector.tensor_reduce(racc[:, 8 + ns : 9 + ns], pr2,
                                        axis=AX.X, op=OP.add)

            nv = stagep.tile([15, 1], F32, tag="nv")
            nc.vector.tensor_reduce(nv, racc, axis=AX.X, op=OP.add)
            ones15 = consts.tile([15, 1], F32, tag="ones15")
            nc.vector.memset(ones15, 1.0)
            psn = ps_small.tile([15, 64 * BL], F32, tag="small")
            nc.tensor.matmul(psn[:1, :1], ones15, nv, start=True, stop=True)
            num11 = work.tile([1, 1], F32, tag="num11", bufs=1)
            nc.vector.tensor_copy(num11, psn[:1, :1])

            # ---- CRF forward scan, linear space ---------------------------
            # p_t = E_t o (Mexp^T p_{t-1}), Mexp = exp(trans)/16 (fp32 matmul).
            # logZ = ln(sum_j p_T) + 511*ln16 + renorm offsets.
            mexp = consts.tile([15, 15], F32, tag="mexp")
            nlog16 = consts.tile([15, 1], F32, tag="nlog16")
            nc.vector.memset(nlog16, -LOG16)
            nc.scalar.activation(mexp, trans, AF.Exp, bias=nlog16)
            ones115 = consts.tile([1, 15], F32, tag="ones115")
            nc.vector.memset(ones115, 1.0)

            # start/end potentials are already folded into logits (hence Et)
            p0 = seqs.tile([15, BL], F32, tag="p0")
            nc.vector.tensor_copy(p0, Et[:, 0:BL])
            off_r = seqs.tile([1, BL], F32, tag="off_r")
            nc.vector.memset(off_r, 0.0)

            pcur = [p0]

            def crf_step(s):
                z = ps_small.tile([15, 64 * BL], F32, tag="small")
                nc.tensor.matmul(z[:, 0:BL], mexp, pcur[0], start=True, stop=True)
                pn = stagep.tile([15, BL], F32, tag="pnew", bufs=4)
                nc.vector.tensor_mul(pn, z[:, 0:BL], Et[:, ds(s * BL, BL)])
                pcur[0] = pn

            def crf_renorm():
                # per-batch max over states -> scale p to ~1, accumulate log
                pt = ps_small.tile([15, 64 * BL], F32, tag="small")
                nc.tensor.transpose(pt[:BL, :15], pcur[0], ident[:15, :15])
                a8 = stagep.tile([BL, 15], F32, tag="crfa8")
                nc.vector.tensor_copy(a8, pt[:BL, :15])
                mx = stagep.tile([BL, 1], F32, tag="crfmx")
                nc.vector.tensor_reduce(mx, a8, axis=AX.X, op=OP.max)
                lmx = stagep.tile([BL, 1], F32, tag="crflmx")
                nc.scalar.activation(lmx, mx, AF.Ln)
                rcp = stagep.tile([BL, 1], F32, tag="crfrcp")
                nc.vector.reciprocal(rcp, mx)
                # transpose [BL,1] -> [1,BL] twice (scale and log-offset)
                ptm = ps_small.tile([15, 64 * BL], F32, tag="small")
                nc.tensor.transpose(ptm[:1, :BL], rcp, ident[:BL, :BL])
                nc.tensor.transpose(ptm[:1, BL : 2 * BL], lmx, ident[:BL, :BL])
                sc = stagep.tile([1, 2 * BL], F32, tag="crfsc")
                nc.vector.tensor_copy(sc, ptm[:1, 0 : 2 * BL])
                nc.vector.tensor_add(off_r, off_r, sc[:, BL : 2 * BL])
                scb = ps_small.tile([15, 64 * BL], F32, tag="small")
                nc.tensor.matmul(scb[:15, 0:BL], ones115, sc[:, 0:BL], start=True, stop=True)
                pn = stagep.tile([15, BL], F32, tag="pnew", bufs=4)
                nc.vector.tensor_mul(pn, pcur[0], scb[:15, 0:BL])
                pcur[0] = pn

            # python-unrolled scan: each step is only 2 tiny instructions
            for s in range(1, T):
                crf_step(s)
                if s % 64 == 0:
                    crf_renorm()

            # ---- denominator + output -------------------------------------
            pse = ps_small.tile([15, 64 * BL], F32, tag="small")
            nc.tensor.matmul(pse[:1, 0:BL], ones15, pcur[0], start=True, stop=True)
            den8 = stagep.tile([1, BL], F32, tag="den8")
            nc.scalar.activation(den8, pse[:1, 0:BL], AF.Ln)
            nc.vector.tensor_add(den8, den8, off_r)
            dsum = stagep.tile([1, 1], F32, tag="dsum")
            nc.vector.tensor_reduce(dsum, den8, axis=AX.X, op=OP.add)
            res = work.tile([1, 1], F32, tag="res", bufs=1)
            # den total = dsum + BL*511*ln16 ; res = den - num
            logc = consts.tile([1, 1], F32, tag="logc")
            nc.vector.memset(logc, float(BL * 511) * LOG16)
            nc.scalar.activation(res, dsum, AF.Identity, bias=logc)
            nc.vector.tensor_sub(res, res, num11)
            nc.sync.dma_start(out=out_ext[:, :], in_=res)

    nc.finalize()
    _BUILD_CACHE["nc"] = nc
    return nc


# ---- host-side input prep ---------------------------------------------------

_GPERM = np.concatenate([np.arange(0, 512), np.arange(768, 1024), np.arange(512, 768)])


def _wih_prep(W, dk_n):
    # lhsT tiles: [p, dk, m, c] = W[gperm[m*128+c], dk*128+p]
    Wp = W[_GPERM]
    return np.ascontiguousarray(
        Wp.reshape(8, 128, dk_n, 128).transpose(3, 2, 0, 1).reshape(128, dk_n * 8 * 128)
    ).astype(np.float16)


def _common_inputs(inputs):
    out = {}
    bias_cols = {}
    for key in ("c0", "c1", "w0", "w1"):
        Wih = np.asarray(inputs[f"{key}_Wih"], np.float32)
        Whh = np.asarray(inputs[f"{key}_Whh"], np.float32)
        bih = np.asarray(inputs[f"{key}_bih"], np.float32)
        bhh = np.asarray(inputs[f"{key}_bhh"], np.float32)
        dk_n = Wih.shape[2] // 128
        for r, sfx in ((0, "f"), (1, "b")):
            out[f"wih_{key}{sfx}"] = _wih_prep(Wih[r], dk_n)
            out[f"whh_{key}{sfx}"] = _wih_prep(Whh[r], 2)
            bb = (bih[r] + bhh[r])[_GPERM]
            bias_cols[f"{key}{sfx}"] = bb.reshape(8, 128).T  # (128, 8)
    out["biasall"] = np.ascontiguousarray(
        np.stack([bias_cols[nm] for nm in DIR_ORDER], axis=1)
    ).astype(np.float32)
    w1 = np.asarray(inputs["cls_w1"], np.float32)  # (512, 1024)
    out["cls1"] = np.ascontiguousarray(
        w1.reshape(4, 128, 8, 128).transpose(3, 2, 0, 1).reshape(128, 8 * 4 * 128)
    ).astype(np.float16)
    out["clsb1"] = np.ascontiguousarray(
        np.asarray(inputs["cls_b1"], np.float32).reshape(4, 128).T
    ).astype(np.float32)
    w2 = np.asarray(inputs["cls_w2"], np.float32)  # (15, 512)
    out["cls2"] = np.ascontiguousarray(
        w2.reshape(15, 4, 128).transpose(2, 1, 0).reshape(128, 4 * 15)
    ).astype(np.float16)
    out["clsb2"] = np.asarray(inputs["cls_b2"], np.float32).reshape(15, 1).copy()
    out["trans"] = np.asarray(inputs["crf_trans"], np.float32).copy()
    out["crfstart"] = np.asarray(inputs["crf_start"], np.float32).reshape(15, 1).copy()
    out["crfend"] = np.asarray(inputs["crf_end"], np.float32).reshape(15, 1).copy()
    return out


def make_in_maps(inputs):
    common = _common_inputs(inputs)
    char_ids = np.asarray(inputs["char_ids"])
    tags = np.asarray(inputs["tags"])
    wemb = np.asarray(inputs["word_embeddings"], np.float32)
    emb = np.asarray(inputs["char_emb_table"], np.float32)
    in_maps = []
    for c in range(NC_N):
        lo, hi = c * BL, (c + 1) * BL
        m = dict(common)
        ce = emb[char_ids[lo:hi]]  # (BL, T, 128)
        m["ceT"] = np.ascontiguousarray(
            ce.transpose(2, 1, 0).reshape(128, 1, TB)
        ).astype(np.float16)
        m["weT"] = np.ascontiguousarray(
            wemb[lo:hi].reshape(BL, T, 6, 128).transpose(3, 2, 1, 0).reshape(128, 6, TB)
        ).astype(np.float16)
        oh = (np.arange(K)[:, None, None] == tags[lo:hi][None]).astype(np.float32)
        m["tagoneT"] = np.ascontiguousarray(oh.transpose(0, 2, 1).reshape(K, TB)).astype(np.float16)
        in_maps.append(m)
    return in_maps


def kernel(**inputs):
    nc = _build_nc()
    in_maps = make_in_maps(inputs)
    res = run_bass_kernel_spmd(nc, in_maps, core_ids=list(range(NC_N)))
    total = sum(float(res.results[c]["out"][0, 0]) for c in range(NC_N))
    return np.float32(total / B)
